# revision 9
# baseline (speedup 1.0000x reference)
"""ConceptNet encoder kernel for 8 Trainium2 NeuronCores (Bass/Tile).

Reference computation:
    emb    = table[tok]                      # [1024, 256]
    logits = emb @ table.T                   # [1024, 100000]
    idx    = top16(softmax(logits))          # softmax monotonic -> top16(logits)
    h      = table[idx]                      # [1024, 16, 256]
    e      = tanh(h @ a) @ b                 # [1024, 16]
    out    = softmax(e) @ h                  # [1024, 256]

Distribution: vocab sharded 8 ways. The similarity matmul runs in
float32r (~bf16 speed, ~12-bit mantissa inputs, fp32 accumulate).
Selection packs (quantized value, slot) into one uint32 key per logit
(scalar-engine quantize + one DVE shift-or pass; a few chunks take a
scalar-shift + gpsimd-add path instead to offload the DVE), takes
per-chunk top-8 with a single MAX8 (no FIND_INDEX8, no index plane),
and AllToAll's one key plane per chunk-group (4 pipelined collectives
overlapped with compute). The vocab-shard strips stream through a ring
of SBUF tiles (chunk-group-outer loop), so there is no bulk-load
startup stall. Each core then merges a top-20 candidate pool for its
own 128 tokens, re-scores the pool exactly in fp32 (rescue), and runs
masked-softmax attention over the pool so exactly the true top-16 get
weight.

kernel(**inputs) takes FULL unsharded inputs, returns FULL [4,256,256] output.
Self-contained: hardcodes all shapes; imports only the system concourse repo.
"""
import os
import sys

if "/opt/trn_rl_repo" not in sys.path:
    sys.path.insert(0, "/opt/trn_rl_repo")

import numpy as np

import concourse.bass as bass
import concourse.bacc as bacc
import concourse.mybir as mybir
import concourse.tile as tile
from concourse import bass_utils
from concourse.masks import make_identity

DT = mybir.dt
ALU = mybir.AluOpType
ACT = mybir.ActivationFunctionType

B, L, V, E, TOPK = 4, 256, 100000, 256, 16
NCORES = 8
NTOK = B * L                 # 1024
TPC = NTOK // NCORES         # 128 tokens per core (merge/attention shard)
VS = V // NCORES             # 12500 vocab rows per core
P = 128
NEG = -3.0e38

CW = 1024                    # similarity chunk width (2 PSUM banks)
CHUNKS = []
_off = 0
while _off < VS:
    CHUNKS.append((_off, min(CW, VS - _off)))
    _off += CW
NCHUNK = len(CHUNKS)         # 13 (12x1024 + 212)
PARTS = [[0, 1, 2], [3, 4, 5], [6, 7, 8], [9, 10, 11, 12]]  # j = 3*part + jl
GP_CHUNKS = {1, 4, 7, 11}    # pack pass runs scalar+gpsimd for these chunks
RING = 6                     # streamed tabT strip ring depth (per kb)
NCP = 32                     # candidate slots per (core, part): <=4 chunks x 8
KP = 20                      # rescue pool size per token
KPAD = 24                    # padded pool for max8 rounds
NGR = KP // 4                # attention 512-wide groups
QSCALE = 1536.0              # logit quantizer scale
QBIAS = 3456.0               # makes qi positive (logits in [-2.25, 2])
KEYSHIFT = 11                # slot bits
KEYBASE = 1 << 30            # keeps key bit patterns in normal-float range
MERGEW = 4 * NCORES * NCP    # 1024

_BUILD_CACHE = {}
LAST_RESULTS = None


def _round12(x):
    """Round fp32 to 12 explicit mantissa bits (round half even)."""
    u = np.ascontiguousarray(x, dtype=np.float32).view(np.uint32)
    shift = np.uint32(11)
    mask = np.uint32((1 << 11) - 1)
    half = np.uint32(1 << 10)
    frac = u & mask
    u2 = u & ~mask
    rnd = (frac > half) | ((frac == half) & (((u2 >> shift) & np.uint32(1)) == 1))
    u2 = u2 + (rnd.astype(np.uint32) << shift)
    return u2.view(np.float32)


def _build():
    nc = bacc.Bacc("TRN2", target_bir_lowering=False, debug=False,
                   enable_asserts=True, num_devices=NCORES)

    tokidx = nc.dram_tensor("tokidx", [NTOK, 1], DT.int32, kind="ExternalInput").ap()
    tok_own = nc.dram_tensor("tok_own", [TPC, 1], DT.int32, kind="ExternalInput").ap()
    table = nc.dram_tensor("table", [V, E], DT.float32, kind="ExternalInput").ap()
    tabTr = nc.dram_tensor("tabTr", [E, VS], DT.float32r, kind="ExternalInput").ap()
    amat = nc.dram_tensor("amat", [E, E], DT.float32, kind="ExternalInput").ap()
    bvec = nc.dram_tensor("bvec", [E, 1], DT.float32, kind="ExternalInput").ap()
    out = nc.dram_tensor("out", [TPC, E], DT.float32, kind="ExternalOutput").ap()

    with tile.TileContext(nc) as tc:
        with tc.tile_pool(name="const", bufs=1) as cpool, \
             tc.tile_pool(name="big", bufs=1) as big, \
             tc.tile_pool(name="work", bufs=2) as work, \
             tc.tile_pool(name="ps_chunk", bufs=3, space="PSUM") as ps_chunk, \
             tc.tile_pool(name="ps_tr", bufs=1, space="PSUM") as ps_tr, \
             tc.tile_pool(name="ps_att", bufs=1, space="PSUM") as ps_att, \
             tc.tile_pool(name="dram", bufs=1, space="DRAM") as dram:

            # ---------------- constants ----------------
            ident = cpool.tile([P, P], DT.float32, tag="ident")
            make_identity(nc, ident)

            iotaK = cpool.tile([P, CW], DT.uint32, tag="iotaK")
            nc.gpsimd.iota(iotaK, pattern=[[1, CW]], base=KEYBASE,
                           channel_multiplier=0)

            def const_col(name, val):
                t = cpool.tile([P, 1], DT.uint32, tag=name, name=name)
                nc.gpsimd.iota(t, pattern=[[0, 1]], base=val, channel_multiplier=0)
                return t

            c_shift = const_col("c_shift", KEYSHIFT)
            c_slotmask = const_col("c_slotmask", (1 << KEYSHIFT) - 1)
            c_8 = const_col("c_8", 8)
            c_5 = const_col("c_5", 5)
            c_7 = const_col("c_7", 7)
            c_3 = const_col("c_3", 3)

            # ---------------- emb gather + f32r transpose ----------------
            embT = [big.tile([P, NTOK], DT.float32r, tag=f"embT{kb}",
                             name=f"embT{kb}")
                    for kb in range(2)]
            for m in range(NCORES):
                ti = work.tile([P, 1], DT.int32, tag="ti")
                nc.sync.dma_start(out=ti, in_=tokidx[m * P:(m + 1) * P, :])
                em = work.tile([P, E], DT.float32, tag="em")
                nc.gpsimd.indirect_dma_start(
                    out=em, out_offset=None, in_=table,
                    in_offset=bass.IndirectOffsetOnAxis(ap=ti[:, :], axis=0))
                for kb in range(2):
                    pt = ps_tr.tile([P, P], DT.float32, tag="tr")
                    nc.tensor.transpose(out=pt, in_=em[:, kb * P:(kb + 1) * P],
                                        identity=ident)
                    nc.vector.tensor_copy(embT[kb][:, m * P:(m + 1) * P], pt)

            # own-token embeddings (fp32, for exact rescue dots)
            ti_own = cpool.tile([P, 1], DT.int32, tag="ti_own")
            nc.sync.dma_start(out=ti_own, in_=tok_own)
            emb_own = cpool.tile([P, E], DT.float32, tag="emb_own")
            nc.gpsimd.indirect_dma_start(
                out=emb_own, out_offset=None, in_=table,
                in_offset=bass.IndirectOffsetOnAxis(ap=ti_own[:, :], axis=0))

            # ---------------- small weights ----------------
            a_sb = []
            for kb in range(2):
                t = cpool.tile([P, E], DT.float32, tag=f"a{kb}", name=f"a{kb}")
                nc.sync.dma_start(out=t, in_=amat[kb * P:(kb + 1) * P, :])
                a_sb.append(t)
            a_r = []
            for kb in range(2):
                t = cpool.tile([P, E], DT.float32r, tag=f"ar{kb}", name=f"ar{kb}")
                nc.vector.tensor_copy(t, a_sb[kb])
                a_r.append(t)
            b_sb = []
            for kb in range(2):
                t = cpool.tile([P, 1], DT.float32, tag=f"b{kb}", name=f"b{kb}")
                nc.sync.dma_start(out=t, in_=bvec[kb * P:(kb + 1) * P, :])
                b_sb.append(t)
            b_r = []
            for kb in range(2):
                t = cpool.tile([P, 1], DT.float32r, tag=f"br{kb}", name=f"br{kb}")
                nc.vector.tensor_copy(t, b_sb[kb])
                b_r.append(t)

            # ---------------- streamed tabT strips ----------------
            strip = {}

            def issue_strips(part):
                for j in PARTS[part]:
                    off, w = CHUNKS[j]
                    for kb in range(2):
                        t = big.tile([P, w], DT.float32r, tag=f"tt{kb}_{j % RING}",
                                     name=f"tt{kb}_{j % RING}")
                        nc.sync.dma_start(out=t, in_=tabTr[kb * P:(kb + 1) * P,
                                                           off:off + w])
                        strip[(kb, j)] = t

            issue_strips(0)

            # ---------------- a2a bounce buffers ----------------
            bounce = [dram.tile([NCORES, TPC, NCP], DT.float32, tag=f"bounce{p}",
                                name=f"bounce{p}")
                      for p in range(4)]
            agg = [dram.tile([NCORES * TPC * NCP, 1], DT.float32, tag=f"agg{p}",
                             name=f"agg{p}")
                   for p in range(4)]
            scd = dram.tile([1, TPC * KP], DT.float32, tag="scd")

            # ---------------- similarity + packed per-chunk top-8 --------
            for part in range(4):
                if part + 1 < 4:
                    issue_strips(part + 1)
                pjs = PARTS[part]
                for m in range(NCORES):
                    cv = work.tile([P, NCP], DT.float32, tag="cv")
                    if len(pjs) < 4:
                        nc.gpsimd.memset(cv[:, len(pjs) * 8:], 0.0)
                    for jl, j in enumerate(pjs):
                        off, w = CHUNKS[j]
                        ps = ps_chunk.tile([P, CW], DT.float32, tag="chunk")
                        for kb in range(2):
                            for h in range((w + 511) // 512):
                                hw = min(512, w - h * 512)
                                nc.tensor.matmul(
                                    ps[:, h * 512:h * 512 + hw],
                                    embT[kb][:, m * P:(m + 1) * P],
                                    strip[(kb, j)][:, h * 512:h * 512 + hw],
                                    start=(kb == 0), stop=(kb == 1))
                        keys = work.tile([P, CW], DT.uint32, tag="keys")
                        if j in GP_CHUNKS:
                            # scalar: quantize + exact *2048 shift; gpsimd: +iota
                            q1 = work.tile([P, CW], DT.int32, tag="q1")
                            nc.scalar.activation(q1[:, :w], ps[:, :w],
                                                 ACT.Copy, scale=QSCALE,
                                                 bias=QBIAS)
                            nc.scalar.activation(keys[:, :w].bitcast(DT.int32),
                                                 q1[:, :w], ACT.Copy,
                                                 scale=float(1 << KEYSHIFT))
                            nc.gpsimd.tensor_tensor(keys[:, :w], keys[:, :w],
                                                    iotaK[:, :w], op=ALU.add)
                        else:
                            # quantize logits -> int (scalar engine reads PSUM)
                            nc.scalar.activation(keys[:, :w].bitcast(DT.int32),
                                                 ps[:, :w], ACT.Copy,
                                                 scale=QSCALE, bias=QBIAS)
                            # key = (qi << 11) | slot | 2^30  (one DVE pass)
                            nc.vector.scalar_tensor_tensor(
                                keys[:, :w], keys[:, :w], c_shift[:, :],
                                iotaK[:, :w],
                                op0=ALU.logical_shift_left, op1=ALU.bitwise_or)
                        nc.vector.max(out=cv[:, jl * 8:(jl + 1) * 8],
                                      in_=keys[:, :w].bitcast(DT.float32))
                    nc.sync.dma_start(out=bounce[part][m, :, :], in_=cv)

                nc.gpsimd.collective_compute(
                    "AllToAll", ALU.bypass,
                    replica_groups=[list(range(NCORES))],
                    ins=[bounce[part][:, :, :].opt()],
                    outs=[agg[part][:, :].opt()],
                )

            # vals[p, part*256 + c*32 + s] = agg[part][(c, p, s)]
            vals = cpool.tile([P, MERGEW], DT.float32, tag="vals")
            for part in range(4):
                agg_v = agg[part][:, :].rearrange("(c p s) o -> c p (s o)",
                                                  c=NCORES, p=TPC)
                for c in range(NCORES):
                    o = part * NCORES * NCP + c * NCP
                    nc.gpsimd.dma_start(out=vals[:, o:o + NCP], in_=agg_v[c])

            # ---------------- merge: top-20 keys + positions -------------
            wk = cpool.tile([P, KPAD], DT.float32, tag="wk")
            wp = cpool.tile([P, KPAD], DT.uint32, tag="wp")
            vals2 = cpool.tile([P, MERGEW], DT.float32, tag="vals2")
            vals3 = cpool.tile([P, MERGEW], DT.float32, tag="vals3")
            nc.vector.max(out=wk[:, 0:8], in_=vals)
            nc.vector.max_index(out=wp[:, 0:8], in_max=wk[:, 0:8], in_values=vals)
            nc.vector.match_replace(out=vals2, in_to_replace=wk[:, 0:8],
                                    in_values=vals, imm_value=0.0)
            nc.vector.max(out=wk[:, 8:16], in_=vals2)
            nc.vector.max_index(out=wp[:, 8:16], in_max=wk[:, 8:16], in_values=vals2)
            nc.vector.match_replace(out=vals3, in_to_replace=wk[:, 8:16],
                                    in_values=vals2, imm_value=0.0)
            nc.vector.max(out=wk[:, 16:24], in_=vals3)
            nc.vector.max_index(out=wp[:, 16:24], in_max=wk[:, 16:24], in_values=vals3)

            # ---------------- decode global vocab indices ----------------
            # pos = part*256 + c*32 + jl*8 + r ; key = (qi<<11)|slot|2^30
            kp = slice(0, KP)
            slot = cpool.tile([P, KP], DT.uint32, tag="slot")
            nc.vector.tensor_scalar(slot, wk[:, kp].bitcast(DT.uint32),
                                    c_slotmask[:, :], None, op0=ALU.bitwise_and)
            prt = cpool.tile([P, KP], DT.uint32, tag="prt")
            nc.vector.tensor_scalar(prt, wp[:, kp], c_8[:, :], None,
                                    op0=ALU.logical_shift_right)
            csrc = cpool.tile([P, KP], DT.uint32, tag="csrc")
            nc.vector.tensor_scalar(csrc, wp[:, kp], c_5[:, :], None,
                                    op0=ALU.logical_shift_right)
            nc.vector.tensor_scalar(csrc, csrc, c_7[:, :], None,
                                    op0=ALU.bitwise_and)
            jl = cpool.tile([P, KP], DT.uint32, tag="jl")
            nc.vector.tensor_scalar(jl, wp[:, kp], c_3[:, :], None,
                                    op0=ALU.logical_shift_right)
            nc.vector.tensor_scalar(jl, jl, c_3[:, :], None,
                                    op0=ALU.bitwise_and)
            # gidx = csrc*12500 + (3*part + jl)*1024 + slot  (< 2^24: fp-exact)
            gidx = cpool.tile([P, KP], DT.uint32, tag="gidx")
            nc.vector.tensor_scalar(gidx, csrc, float(VS), None, op0=ALU.mult)
            t2 = cpool.tile([P, KP], DT.uint32, tag="t2")
            nc.vector.tensor_scalar(t2, prt, 3.0 * CW, None, op0=ALU.mult)
            nc.vector.tensor_tensor(gidx, gidx, t2, op=ALU.add)
            nc.vector.tensor_scalar(t2, jl, float(CW), None, op0=ALU.mult)
            nc.vector.tensor_tensor(gidx, gidx, t2, op=ALU.add)
            nc.vector.tensor_tensor(gidx, gidx, slot, op=ALU.add)

            # ---------------- rescue: gather h + exact fp32 dots ---------
            hk = [cpool.tile([P, E], DT.float32, tag=f"h{k}", name=f"h{k}")
                  for k in range(KP)]
            gidx_i = gidx[:, :].bitcast(DT.int32)
            for k in range(KP):
                nc.gpsimd.indirect_dma_start(
                    out=hk[k], out_offset=None, in_=table,
                    in_offset=bass.IndirectOffsetOnAxis(ap=gidx_i[:, k:k + 1],
                                                        axis=0))
            d = cpool.tile([P, KPAD], DT.float32, tag="d")
            nc.vector.memset(d[:, KP:], NEG)
            prod = cpool.tile([P, E], DT.float32, tag="prod")
            for k in range(KP):
                nc.vector.scalar_tensor_tensor(
                    prod, hk[k], 1.0, emb_own,
                    op0=ALU.mult, op1=ALU.mult, accum_out=d[:, k:k + 1])

            # 16th largest exact dot -> threshold mask
            t8a = cpool.tile([P, 8], DT.float32, tag="t8a")
            t8b = cpool.tile([P, 8], DT.float32, tag="t8b")
            d2 = cpool.tile([P, KPAD], DT.float32, tag="d2")
            nc.vector.max(out=t8a, in_=d)
            nc.vector.match_replace(out=d2, in_to_replace=t8a, in_values=d,
                                    imm_value=NEG)
            nc.vector.max(out=t8b, in_=d2)
            # maskp = (1[d >= thr16] - 1) * 1e9   (0 for kept, -1e9 for dropped)
            maskp = cpool.tile([P, KP], DT.float32, tag="maskp")
            nc.vector.tensor_scalar(maskp, d[:, :KP], t8b[:, 7:8], None,
                                    op0=ALU.is_ge)
            nc.vector.tensor_scalar(maskp, maskp, -1.0, 1.0e9,
                                    op0=ALU.add, op1=ALU.mult)

            # ---------------- attention over the 20-candidate pool -------
            # hT chunks: n = k*128 + t, grouped 4 k's per 512-wide chunk
            for g in range(NGR):
                hTs = [work.tile([P, 512], DT.float32r, tag=f"hTs{kb}",
                                 name=f"hTs{kb}")
                       for kb in range(2)]
                for kk in range(4):
                    k = g * 4 + kk
                    for kb in range(2):
                        pt = ps_tr.tile([P, P], DT.float32, tag="tr")
                        nc.tensor.transpose(out=pt,
                                            in_=hk[k][:, kb * P:(kb + 1) * P],
                                            identity=ident)
                        nc.vector.tensor_copy(hTs[kb][:, kk * P:(kk + 1) * P], pt)
                tanhTs = [work.tile([P, 512], DT.float32r, tag=f"tanhTs{eb}",
                                    name=f"tanhTs{eb}")
                          for eb in range(2)]
                for eb in range(2):
                    pta = ps_att.tile([P, 512], DT.float32, tag="att")
                    for kb in range(2):
                        nc.tensor.matmul(pta, a_r[kb][:, eb * P:(eb + 1) * P],
                                         hTs[kb], start=(kb == 0), stop=(kb == 1))
                    nc.scalar.activation(tanhTs[eb], pta, ACT.Tanh)
                psc = ps_att.tile([1, 512], DT.float32, tag="att", name="psc")
                for eb in range(2):
                    nc.tensor.matmul(psc, b_r[eb], tanhTs[eb],
                                     start=(eb == 0), stop=(eb == 1))
                scs = work.tile([1, 512], DT.float32, tag="scs")
                nc.vector.tensor_copy(scs, psc)
                nc.sync.dma_start(out=scd[:, g * 512:(g + 1) * 512], in_=scs)

            # scores [t, k] <- scd[k*128 + t]
            sct = cpool.tile([P, KP], DT.float32, tag="sct")
            nc.sync.dma_start(out=sct,
                              in_=scd[:, :].rearrange("o (k t) -> (o t) k", t=TPC))

            # masked softmax over k
            nc.vector.tensor_tensor(sct, sct, maskp, op=ALU.add)
            mx = cpool.tile([P, 1], DT.float32, tag="mx")
            nc.vector.reduce_max(mx, sct, axis=mybir.AxisListType.X)
            negmx = cpool.tile([P, 1], DT.float32, tag="negmx")
            nc.vector.tensor_scalar(negmx, mx, -1.0, None, op0=ALU.mult)
            ex = cpool.tile([P, KP], DT.float32, tag="ex")
            nc.scalar.activation(ex, sct, ACT.Exp, bias=negmx[:, :], scale=1.0)
            sm = cpool.tile([P, 1], DT.float32, tag="sm")
            nc.vector.reduce_sum(sm, ex, axis=mybir.AxisListType.X)
            rc = cpool.tile([P, 1], DT.float32, tag="rc")
            nc.vector.reciprocal(rc, sm)
            att = cpool.tile([P, KP], DT.float32, tag="att_w")
            nc.vector.tensor_scalar(att, ex, rc[:, :], None, op0=ALU.mult)

            # out[t, e] = sum_k att[t,k] * h[t,k,e]
            acc = cpool.tile([P, E], DT.float32, tag="acc")
            nc.vector.memset(acc, 0.0)
            for k in range(KP):
                nc.vector.scalar_tensor_tensor(
                    acc, hk[k], att[:, k:k + 1], acc,
                    op0=ALU.mult, op1=ALU.add)
            nc.sync.dma_start(out=out, in_=acc)

    nc.compile()
    return nc


def get_nc():
    if "v3" not in _BUILD_CACHE:
        _BUILD_CACHE["v3"] = _build()
    return _BUILD_CACHE["v3"]


def kernel(conceptnet_text_vec, table, a, b, topk=16, **_ignored):
    global LAST_RESULTS
    assert int(topk) == TOPK
    tok = np.asarray(conceptnet_text_vec).reshape(NTOK, 1).astype(np.int32)
    table = np.ascontiguousarray(np.asarray(table, dtype=np.float32))
    a = np.ascontiguousarray(np.asarray(a, dtype=np.float32))
    b = np.ascontiguousarray(np.asarray(b, dtype=np.float32)).reshape(E, 1)
    tabT_r = _round12(np.ascontiguousarray(table.T))   # [E, V], f32r-rounded

    nc = get_nc()
    in_maps = []
    for c in range(NCORES):
        in_maps.append({
            "tokidx": tok,
            "tok_own": np.ascontiguousarray(tok[c * TPC:(c + 1) * TPC]),
            "table": table,
            "tabTr": np.ascontiguousarray(tabT_r[:, c * VS:(c + 1) * VS]),
            "amat": a,
            "bvec": b,
        })
    trace = bool(int(os.environ.get("CN_TRACE", "0")))
    res = bass_utils.run_bass_kernel_spmd(nc, in_maps, core_ids=list(range(NCORES)),
                                          trace=trace)
    LAST_RESULTS = res
    outp = np.concatenate([res.results[c]["out"] for c in range(NCORES)], axis=0)
    return outp.reshape(B, L, E)


# revision 11
# speedup vs baseline: 1.0114x; 1.0114x over previous
"""ConceptNet encoder kernel for 8 Trainium2 NeuronCores (Bass/Tile).

Reference computation:
    emb    = table[tok]                      # [1024, 256]
    logits = emb @ table.T                   # [1024, 100000]
    idx    = top16(softmax(logits))          # softmax monotonic -> top16(logits)
    h      = table[idx]                      # [1024, 16, 256]
    e      = tanh(h @ a) @ b                 # [1024, 16]
    out    = softmax(e) @ h                  # [1024, 256]

Distribution: vocab sharded 8 ways. The similarity matmul runs in
float32r (~bf16 speed, ~12-bit mantissa inputs, fp32 accumulate).
Selection packs (quantized value, slot) into one uint32 key per logit
(scalar-engine quantize + one DVE shift-or pass; a few chunks take a
scalar-shift + gpsimd-add path instead to offload the DVE), takes
per-chunk top-8 with a single MAX8 (no FIND_INDEX8, no index plane),
and AllToAll's one key plane per chunk-group (4 pipelined collectives
overlapped with compute). The vocab-shard strips stream through a ring
of SBUF tiles (chunk-group-outer loop), so there is no bulk-load
startup stall. Each core then merges a top-20 candidate pool for its
own 128 tokens, re-scores the pool exactly in fp32 (rescue), and runs
masked-softmax attention over the pool so exactly the true top-16 get
weight.

kernel(**inputs) takes FULL unsharded inputs, returns FULL [4,256,256] output.
Self-contained: hardcodes all shapes; imports only the system concourse repo.
"""
import os
import sys

if "/opt/trn_rl_repo" not in sys.path:
    sys.path.insert(0, "/opt/trn_rl_repo")

import numpy as np

import concourse.bass as bass
import concourse.bacc as bacc
import concourse.mybir as mybir
import concourse.tile as tile
from concourse import bass_utils
from concourse.masks import make_identity

DT = mybir.dt
ALU = mybir.AluOpType
ACT = mybir.ActivationFunctionType

B, L, V, E, TOPK = 4, 256, 100000, 256, 16
NCORES = 8
NTOK = B * L                 # 1024
TPC = NTOK // NCORES         # 128 tokens per core (merge/attention shard)
VS = V // NCORES             # 12500 vocab rows per core
P = 128
NEG = -3.0e38

CW = 1024                    # similarity chunk width (2 PSUM banks)
CHUNKS = []
_off = 0
while _off < VS:
    CHUNKS.append((_off, min(CW, VS - _off)))
    _off += CW
NCHUNK = len(CHUNKS)         # 13 (12x1024 + 212)
PARTS = [[0, 1, 2], [3, 4, 5], [6, 7, 8], [9, 10, 11, 12]]  # j = 3*part + jl
GP_CHUNKS = {1, 4, 7, 11}    # pack pass runs scalar+gpsimd for these chunks
RING = 10                    # streamed tabT strip ring depth (per kb)
NCP = 32                     # candidate slots per (core, part): <=4 chunks x 8
KP = 20                      # rescue pool size per token
KPAD = 24                    # padded pool for max8 rounds
NGR = KP // 4                # attention 512-wide groups
QSCALE = 1536.0              # logit quantizer scale
QBIAS = 3456.0               # makes qi positive (logits in [-2.25, 2])
KEYSHIFT = 11                # slot bits
KEYBASE = 1 << 30            # keeps key bit patterns in normal-float range
MERGEW = 4 * NCORES * NCP    # 1024

_BUILD_CACHE = {}
LAST_RESULTS = None


def _round12(x):
    """Round fp32 to 12 explicit mantissa bits (round half even)."""
    u = np.ascontiguousarray(x, dtype=np.float32).view(np.uint32)
    shift = np.uint32(11)
    mask = np.uint32((1 << 11) - 1)
    half = np.uint32(1 << 10)
    frac = u & mask
    u2 = u & ~mask
    rnd = (frac > half) | ((frac == half) & (((u2 >> shift) & np.uint32(1)) == 1))
    u2 = u2 + (rnd.astype(np.uint32) << shift)
    return u2.view(np.float32)


def _build():
    nc = bacc.Bacc("TRN2", target_bir_lowering=False, debug=False,
                   enable_asserts=True, num_devices=NCORES)

    tokidx = nc.dram_tensor("tokidx", [NTOK, 1], DT.int32, kind="ExternalInput").ap()
    tok_own = nc.dram_tensor("tok_own", [TPC, 1], DT.int32, kind="ExternalInput").ap()
    table = nc.dram_tensor("table", [V, E], DT.float32, kind="ExternalInput").ap()
    tabTr = nc.dram_tensor("tabTr", [E, VS], DT.float32r, kind="ExternalInput").ap()
    amat = nc.dram_tensor("amat", [E, E], DT.float32, kind="ExternalInput").ap()
    bvec = nc.dram_tensor("bvec", [E, 1], DT.float32, kind="ExternalInput").ap()
    out = nc.dram_tensor("out", [TPC, E], DT.float32, kind="ExternalOutput").ap()

    with tile.TileContext(nc) as tc:
        with tc.tile_pool(name="const", bufs=1) as cpool, \
             tc.tile_pool(name="big", bufs=1) as big, \
             tc.tile_pool(name="work", bufs=2) as work, \
             tc.tile_pool(name="ps_chunk", bufs=3, space="PSUM") as ps_chunk, \
             tc.tile_pool(name="ps_tr", bufs=1, space="PSUM") as ps_tr, \
             tc.tile_pool(name="ps_att", bufs=1, space="PSUM") as ps_att, \
             tc.tile_pool(name="dram", bufs=1, space="DRAM") as dram:

            # ---------------- constants ----------------
            ident = cpool.tile([P, P], DT.float32, tag="ident")
            make_identity(nc, ident)

            iotaK = cpool.tile([P, CW], DT.uint32, tag="iotaK")
            nc.gpsimd.iota(iotaK, pattern=[[1, CW]], base=KEYBASE,
                           channel_multiplier=0)

            def const_col(name, val):
                t = cpool.tile([P, 1], DT.uint32, tag=name, name=name)
                nc.gpsimd.iota(t, pattern=[[0, 1]], base=val, channel_multiplier=0)
                return t

            c_shift = const_col("c_shift", KEYSHIFT)
            c_slotmask = const_col("c_slotmask", (1 << KEYSHIFT) - 1)
            c_8 = const_col("c_8", 8)
            c_5 = const_col("c_5", 5)
            c_7 = const_col("c_7", 7)
            c_3 = const_col("c_3", 3)

            # ---------------- emb gather + f32r transpose ----------------
            embT = [big.tile([P, NTOK], DT.float32r, tag=f"embT{kb}",
                             name=f"embT{kb}")
                    for kb in range(2)]
            for m in range(NCORES):
                ti = work.tile([P, 1], DT.int32, tag="ti")
                nc.sync.dma_start(out=ti, in_=tokidx[m * P:(m + 1) * P, :])
                em = work.tile([P, E], DT.float32, tag="em")
                nc.gpsimd.indirect_dma_start(
                    out=em, out_offset=None, in_=table,
                    in_offset=bass.IndirectOffsetOnAxis(ap=ti[:, :], axis=0))
                for kb in range(2):
                    pt = ps_tr.tile([P, P], DT.float32, tag="tr")
                    nc.tensor.transpose(out=pt, in_=em[:, kb * P:(kb + 1) * P],
                                        identity=ident)
                    nc.vector.tensor_copy(embT[kb][:, m * P:(m + 1) * P], pt)

            # own-token embeddings (fp32, for exact rescue dots)
            ti_own = cpool.tile([P, 1], DT.int32, tag="ti_own")
            nc.sync.dma_start(out=ti_own, in_=tok_own)
            emb_own = cpool.tile([P, E], DT.float32, tag="emb_own")
            nc.gpsimd.indirect_dma_start(
                out=emb_own, out_offset=None, in_=table,
                in_offset=bass.IndirectOffsetOnAxis(ap=ti_own[:, :], axis=0))

            # ---------------- small weights ----------------
            a_sb = []
            for kb in range(2):
                t = cpool.tile([P, E], DT.float32, tag=f"a{kb}", name=f"a{kb}")
                nc.sync.dma_start(out=t, in_=amat[kb * P:(kb + 1) * P, :])
                a_sb.append(t)
            a_r = []
            for kb in range(2):
                t = cpool.tile([P, E], DT.float32r, tag=f"ar{kb}", name=f"ar{kb}")
                nc.vector.tensor_copy(t, a_sb[kb])
                a_r.append(t)
            b_sb = []
            for kb in range(2):
                t = cpool.tile([P, 1], DT.float32, tag=f"b{kb}", name=f"b{kb}")
                nc.sync.dma_start(out=t, in_=bvec[kb * P:(kb + 1) * P, :])
                b_sb.append(t)
            b_r = []
            for kb in range(2):
                t = cpool.tile([P, 1], DT.float32r, tag=f"br{kb}", name=f"br{kb}")
                nc.vector.tensor_copy(t, b_sb[kb])
                b_r.append(t)

            # ---------------- streamed tabT strips ----------------
            strip = {}

            def issue_strips(part):
                for j in PARTS[part]:
                    off, w = CHUNKS[j]
                    for kb in range(2):
                        t = big.tile([P, w], DT.float32r, tag=f"tt{kb}_{j % RING}",
                                     name=f"tt{kb}_{j % RING}")
                        nc.sync.dma_start(out=t, in_=tabTr[kb * P:(kb + 1) * P,
                                                           off:off + w])
                        strip[(kb, j)] = t

            issue_strips(0)
            issue_strips(1)

            # ---------------- a2a bounce buffers ----------------
            bounce = [dram.tile([NCORES, TPC, NCP], DT.float32, tag=f"bounce{p}",
                                name=f"bounce{p}")
                      for p in range(4)]
            agg = [dram.tile([NCORES * TPC * NCP, 1], DT.float32, tag=f"agg{p}",
                             name=f"agg{p}")
                   for p in range(4)]
            scd = dram.tile([1, TPC * KP], DT.float32, tag="scd")

            # ---------------- similarity + packed per-chunk top-8 --------
            for part in range(4):
                if part + 2 < 4:
                    issue_strips(part + 2)
                pjs = PARTS[part]
                for m in range(NCORES):
                    cv = work.tile([P, NCP], DT.float32, tag="cv")
                    if len(pjs) < 4:
                        nc.gpsimd.memset(cv[:, len(pjs) * 8:], 0.0)
                    for jl, j in enumerate(pjs):
                        off, w = CHUNKS[j]
                        ps = ps_chunk.tile([P, CW], DT.float32, tag="chunk")
                        for kb in range(2):
                            for h in range((w + 511) // 512):
                                hw = min(512, w - h * 512)
                                nc.tensor.matmul(
                                    ps[:, h * 512:h * 512 + hw],
                                    embT[kb][:, m * P:(m + 1) * P],
                                    strip[(kb, j)][:, h * 512:h * 512 + hw],
                                    start=(kb == 0), stop=(kb == 1))
                        keys = work.tile([P, CW], DT.uint32, tag="keys")
                        if j in GP_CHUNKS:
                            # scalar: quantize + exact *2048 shift; gpsimd: +iota
                            q1 = work.tile([P, CW], DT.int32, tag="q1")
                            nc.scalar.activation(q1[:, :w], ps[:, :w],
                                                 ACT.Copy, scale=QSCALE,
                                                 bias=QBIAS)
                            nc.scalar.activation(keys[:, :w].bitcast(DT.int32),
                                                 q1[:, :w], ACT.Copy,
                                                 scale=float(1 << KEYSHIFT))
                            nc.gpsimd.tensor_tensor(keys[:, :w], keys[:, :w],
                                                    iotaK[:, :w], op=ALU.add)
                        else:
                            # quantize logits -> int (scalar engine reads PSUM)
                            nc.scalar.activation(keys[:, :w].bitcast(DT.int32),
                                                 ps[:, :w], ACT.Copy,
                                                 scale=QSCALE, bias=QBIAS)
                            # key = (qi << 11) | slot | 2^30  (one DVE pass)
                            nc.vector.scalar_tensor_tensor(
                                keys[:, :w], keys[:, :w], c_shift[:, :],
                                iotaK[:, :w],
                                op0=ALU.logical_shift_left, op1=ALU.bitwise_or)
                        nc.vector.max(out=cv[:, jl * 8:(jl + 1) * 8],
                                      in_=keys[:, :w].bitcast(DT.float32))
                    nc.sync.dma_start(out=bounce[part][m, :, :], in_=cv)

                nc.gpsimd.collective_compute(
                    "AllToAll", ALU.bypass,
                    replica_groups=[list(range(NCORES))],
                    ins=[bounce[part][:, :, :].opt()],
                    outs=[agg[part][:, :].opt()],
                )

            # vals[p, part*256 + c*32 + s] = agg[part][(c, p, s)]
            vals = cpool.tile([P, MERGEW], DT.float32, tag="vals")
            for part in range(4):
                agg_v = agg[part][:, :].rearrange("(c p s) o -> c p (s o)",
                                                  c=NCORES, p=TPC)
                for c in range(NCORES):
                    o = part * NCORES * NCP + c * NCP
                    nc.gpsimd.dma_start(out=vals[:, o:o + NCP], in_=agg_v[c])

            # ---------------- merge: top-20 keys + positions -------------
            wk = cpool.tile([P, KPAD], DT.float32, tag="wk")
            wp = cpool.tile([P, KPAD], DT.uint32, tag="wp")
            vals2 = cpool.tile([P, MERGEW], DT.float32, tag="vals2")
            vals3 = cpool.tile([P, MERGEW], DT.float32, tag="vals3")
            nc.vector.max(out=wk[:, 0:8], in_=vals)
            nc.vector.max_index(out=wp[:, 0:8], in_max=wk[:, 0:8], in_values=vals)
            nc.vector.match_replace(out=vals2, in_to_replace=wk[:, 0:8],
                                    in_values=vals, imm_value=0.0)
            nc.vector.max(out=wk[:, 8:16], in_=vals2)
            nc.vector.max_index(out=wp[:, 8:16], in_max=wk[:, 8:16], in_values=vals2)
            nc.vector.match_replace(out=vals3, in_to_replace=wk[:, 8:16],
                                    in_values=vals2, imm_value=0.0)
            nc.vector.max(out=wk[:, 16:24], in_=vals3)
            nc.vector.max_index(out=wp[:, 16:24], in_max=wk[:, 16:24], in_values=vals3)

            # ---------------- decode global vocab indices ----------------
            # pos = part*256 + c*32 + jl*8 + r ; key = (qi<<11)|slot|2^30
            kp = slice(0, KP)
            slot = cpool.tile([P, KP], DT.uint32, tag="slot")
            nc.vector.tensor_scalar(slot, wk[:, kp].bitcast(DT.uint32),
                                    c_slotmask[:, :], None, op0=ALU.bitwise_and)
            prt = cpool.tile([P, KP], DT.uint32, tag="prt")
            nc.vector.tensor_scalar(prt, wp[:, kp], c_8[:, :], None,
                                    op0=ALU.logical_shift_right)
            csrc = cpool.tile([P, KP], DT.uint32, tag="csrc")
            nc.vector.tensor_scalar(csrc, wp[:, kp], c_5[:, :], None,
                                    op0=ALU.logical_shift_right)
            nc.vector.tensor_scalar(csrc, csrc, c_7[:, :], None,
                                    op0=ALU.bitwise_and)
            jl = cpool.tile([P, KP], DT.uint32, tag="jl")
            nc.vector.tensor_scalar(jl, wp[:, kp], c_3[:, :], None,
                                    op0=ALU.logical_shift_right)
            nc.vector.tensor_scalar(jl, jl, c_3[:, :], None,
                                    op0=ALU.bitwise_and)
            # gidx = csrc*12500 + (3*part + jl)*1024 + slot  (< 2^24: fp-exact)
            gidx = cpool.tile([P, KP], DT.uint32, tag="gidx")
            nc.vector.tensor_scalar(gidx, csrc, float(VS), None, op0=ALU.mult)
            t2 = cpool.tile([P, KP], DT.uint32, tag="t2")
            nc.vector.tensor_scalar(t2, prt, 3.0 * CW, None, op0=ALU.mult)
            nc.vector.tensor_tensor(gidx, gidx, t2, op=ALU.add)
            nc.vector.tensor_scalar(t2, jl, float(CW), None, op0=ALU.mult)
            nc.vector.tensor_tensor(gidx, gidx, t2, op=ALU.add)
            nc.vector.tensor_tensor(gidx, gidx, slot, op=ALU.add)

            # ---------------- rescue: gather h + exact fp32 dots ---------
            hk = [cpool.tile([P, E], DT.float32, tag=f"h{k}", name=f"h{k}")
                  for k in range(KP)]
            gidx_i = gidx[:, :].bitcast(DT.int32)
            for k in range(KP):
                nc.gpsimd.indirect_dma_start(
                    out=hk[k], out_offset=None, in_=table,
                    in_offset=bass.IndirectOffsetOnAxis(ap=gidx_i[:, k:k + 1],
                                                        axis=0))
            d = cpool.tile([P, KPAD], DT.float32, tag="d")
            nc.vector.memset(d[:, KP:], NEG)
            prod = cpool.tile([P, E], DT.float32, tag="prod")
            for k in range(KP):
                nc.vector.scalar_tensor_tensor(
                    prod, hk[k], 1.0, emb_own,
                    op0=ALU.mult, op1=ALU.mult, accum_out=d[:, k:k + 1])

            # 16th largest exact dot -> threshold mask
            t8a = cpool.tile([P, 8], DT.float32, tag="t8a")
            t8b = cpool.tile([P, 8], DT.float32, tag="t8b")
            d2 = cpool.tile([P, KPAD], DT.float32, tag="d2")
            nc.vector.max(out=t8a, in_=d)
            nc.vector.match_replace(out=d2, in_to_replace=t8a, in_values=d,
                                    imm_value=NEG)
            nc.vector.max(out=t8b, in_=d2)
            # maskp = (1[d >= thr16] - 1) * 1e9   (0 for kept, -1e9 for dropped)
            maskp = cpool.tile([P, KP], DT.float32, tag="maskp")
            nc.vector.tensor_scalar(maskp, d[:, :KP], t8b[:, 7:8], None,
                                    op0=ALU.is_ge)
            nc.vector.tensor_scalar(maskp, maskp, -1.0, 1.0e9,
                                    op0=ALU.add, op1=ALU.mult)

            # ---------------- attention over the 20-candidate pool -------
            # hT chunks: n = k*128 + t, grouped 4 k's per 512-wide chunk
            for g in range(NGR):
                hTs = [work.tile([P, 512], DT.float32r, tag=f"hTs{kb}",
                                 name=f"hTs{kb}")
                       for kb in range(2)]
                for kk in range(4):
                    k = g * 4 + kk
                    for kb in range(2):
                        pt = ps_tr.tile([P, P], DT.float32, tag="tr")
                        nc.tensor.transpose(out=pt,
                                            in_=hk[k][:, kb * P:(kb + 1) * P],
                                            identity=ident)
                        nc.vector.tensor_copy(hTs[kb][:, kk * P:(kk + 1) * P], pt)
                tanhTs = [work.tile([P, 512], DT.float32r, tag=f"tanhTs{eb}",
                                    name=f"tanhTs{eb}")
                          for eb in range(2)]
                for eb in range(2):
                    pta = ps_att.tile([P, 512], DT.float32, tag="att")
                    for kb in range(2):
                        nc.tensor.matmul(pta, a_r[kb][:, eb * P:(eb + 1) * P],
                                         hTs[kb], start=(kb == 0), stop=(kb == 1))
                    nc.scalar.activation(tanhTs[eb], pta, ACT.Tanh)
                psc = ps_att.tile([1, 512], DT.float32, tag="att", name="psc")
                for eb in range(2):
                    nc.tensor.matmul(psc, b_r[eb], tanhTs[eb],
                                     start=(eb == 0), stop=(eb == 1))
                scs = work.tile([1, 512], DT.float32, tag="scs")
                nc.vector.tensor_copy(scs, psc)
                nc.sync.dma_start(out=scd[:, g * 512:(g + 1) * 512], in_=scs)

            # scores [t, k] <- scd[k*128 + t]
            sct = cpool.tile([P, KP], DT.float32, tag="sct")
            nc.sync.dma_start(out=sct,
                              in_=scd[:, :].rearrange("o (k t) -> (o t) k", t=TPC))

            # masked softmax over k
            nc.vector.tensor_tensor(sct, sct, maskp, op=ALU.add)
            mx = cpool.tile([P, 1], DT.float32, tag="mx")
            nc.vector.reduce_max(mx, sct, axis=mybir.AxisListType.X)
            negmx = cpool.tile([P, 1], DT.float32, tag="negmx")
            nc.vector.tensor_scalar(negmx, mx, -1.0, None, op0=ALU.mult)
            ex = cpool.tile([P, KP], DT.float32, tag="ex")
            nc.scalar.activation(ex, sct, ACT.Exp, bias=negmx[:, :], scale=1.0)
            sm = cpool.tile([P, 1], DT.float32, tag="sm")
            nc.vector.reduce_sum(sm, ex, axis=mybir.AxisListType.X)
            rc = cpool.tile([P, 1], DT.float32, tag="rc")
            nc.vector.reciprocal(rc, sm)
            att = cpool.tile([P, KP], DT.float32, tag="att_w")
            nc.vector.tensor_scalar(att, ex, rc[:, :], None, op0=ALU.mult)

            # out[t, e] = sum_k att[t,k] * h[t,k,e]
            acc = cpool.tile([P, E], DT.float32, tag="acc")
            nc.vector.memset(acc, 0.0)
            for k in range(KP):
                nc.vector.scalar_tensor_tensor(
                    acc, hk[k], att[:, k:k + 1], acc,
                    op0=ALU.mult, op1=ALU.add)
            nc.sync.dma_start(out=out, in_=acc)

    nc.compile()
    return nc


def get_nc():
    if "v3" not in _BUILD_CACHE:
        _BUILD_CACHE["v3"] = _build()
    return _BUILD_CACHE["v3"]


def kernel(conceptnet_text_vec, table, a, b, topk=16, **_ignored):
    global LAST_RESULTS
    assert int(topk) == TOPK
    tok = np.asarray(conceptnet_text_vec).reshape(NTOK, 1).astype(np.int32)
    table = np.ascontiguousarray(np.asarray(table, dtype=np.float32))
    a = np.ascontiguousarray(np.asarray(a, dtype=np.float32))
    b = np.ascontiguousarray(np.asarray(b, dtype=np.float32)).reshape(E, 1)
    tabT_r = _round12(np.ascontiguousarray(table.T))   # [E, V], f32r-rounded

    nc = get_nc()
    in_maps = []
    for c in range(NCORES):
        in_maps.append({
            "tokidx": tok,
            "tok_own": np.ascontiguousarray(tok[c * TPC:(c + 1) * TPC]),
            "table": table,
            "tabTr": np.ascontiguousarray(tabT_r[:, c * VS:(c + 1) * VS]),
            "amat": a,
            "bvec": b,
        })
    trace = bool(int(os.environ.get("CN_TRACE", "0")))
    res = bass_utils.run_bass_kernel_spmd(nc, in_maps, core_ids=list(range(NCORES)),
                                          trace=trace)
    LAST_RESULTS = res
    outp = np.concatenate([res.results[c]["out"] for c in range(NCORES)], axis=0)
    return outp.reshape(B, L, E)


# revision 12
# speedup vs baseline: 1.0162x; 1.0048x over previous
"""ConceptNet encoder kernel for 8 Trainium2 NeuronCores (Bass/Tile).

Reference computation:
    emb    = table[tok]                      # [1024, 256]
    logits = emb @ table.T                   # [1024, 100000]
    idx    = top16(softmax(logits))          # softmax monotonic -> top16(logits)
    h      = table[idx]                      # [1024, 16, 256]
    e      = tanh(h @ a) @ b                 # [1024, 16]
    out    = softmax(e) @ h                  # [1024, 256]

Distribution: vocab sharded 8 ways. The similarity matmul runs in
float32r (~bf16 speed, ~12-bit mantissa inputs, fp32 accumulate).
Selection packs (quantized value, slot) into one uint32 key per logit
(scalar-engine quantize + one DVE shift-or pass; a few chunks take a
scalar-shift + gpsimd-add path instead to offload the DVE), takes
per-chunk top-8 with a single MAX8 (no FIND_INDEX8, no index plane),
and AllToAll's one key plane per chunk-group (4 pipelined collectives
overlapped with compute). The vocab-shard strips stream through a ring
of SBUF tiles (chunk-group-outer loop), so there is no bulk-load
startup stall. Each core then merges a top-20 candidate pool for its
own 128 tokens, re-scores the pool exactly in fp32 (rescue), and runs
masked-softmax attention over the pool so exactly the true top-16 get
weight.

kernel(**inputs) takes FULL unsharded inputs, returns FULL [4,256,256] output.
Self-contained: hardcodes all shapes; imports only the system concourse repo.
"""
import os
import sys

if "/opt/trn_rl_repo" not in sys.path:
    sys.path.insert(0, "/opt/trn_rl_repo")

import numpy as np

import concourse.bass as bass
import concourse.bacc as bacc
import concourse.mybir as mybir
import concourse.tile as tile
from concourse import bass_utils
from concourse.masks import make_identity

DT = mybir.dt
ALU = mybir.AluOpType
ACT = mybir.ActivationFunctionType

B, L, V, E, TOPK = 4, 256, 100000, 256, 16
NCORES = 8
NTOK = B * L                 # 1024
TPC = NTOK // NCORES         # 128 tokens per core (merge/attention shard)
VS = V // NCORES             # 12500 vocab rows per core
P = 128
NEG = -3.0e38

CW = 1024                    # similarity chunk width (2 PSUM banks)
CHUNKS = []
_off = 0
while _off < VS:
    CHUNKS.append((_off, min(CW, VS - _off)))
    _off += CW
NCHUNK = len(CHUNKS)         # 13 (12x1024 + 212)
PARTS = [[0, 1, 2], [3, 4, 5], [6, 7, 8], [9, 10, 11, 12]]  # j = 3*part + jl
GP_CHUNKS = {1, 4, 7, 11}    # pack pass runs scalar+gpsimd for these chunks
RING = 10                    # streamed tabT strip ring depth (per kb)
NCP = 32                     # candidate slots per (core, part): <=4 chunks x 8
KP = 20                      # rescue pool size per token
KPAD = 24                    # padded pool for max8 rounds
NGR = KP // 4                # attention 512-wide groups
QSCALE = 1536.0              # logit quantizer scale
QBIAS = 3456.0               # makes qi positive (logits in [-2.25, 2])
KEYSHIFT = 11                # slot bits
KEYBASE = 1 << 30            # keeps key bit patterns in normal-float range
MERGEW = 4 * NCORES * NCP    # 1024

_BUILD_CACHE = {}
LAST_RESULTS = None


def _round12(x):
    """Round fp32 to 12 explicit mantissa bits (round half even)."""
    u = np.ascontiguousarray(x, dtype=np.float32).view(np.uint32)
    shift = np.uint32(11)
    mask = np.uint32((1 << 11) - 1)
    half = np.uint32(1 << 10)
    frac = u & mask
    u2 = u & ~mask
    rnd = (frac > half) | ((frac == half) & (((u2 >> shift) & np.uint32(1)) == 1))
    u2 = u2 + (rnd.astype(np.uint32) << shift)
    return u2.view(np.float32)


def _build():
    nc = bacc.Bacc("TRN2", target_bir_lowering=False, debug=False,
                   enable_asserts=True, num_devices=NCORES)

    tokidx = nc.dram_tensor("tokidx", [NTOK, 1], DT.int32, kind="ExternalInput").ap()
    tok_own = nc.dram_tensor("tok_own", [TPC, 1], DT.int32, kind="ExternalInput").ap()
    table = nc.dram_tensor("table", [V, E], DT.float32, kind="ExternalInput").ap()
    tabTr = nc.dram_tensor("tabTr", [E, VS], DT.float32r, kind="ExternalInput").ap()
    amat = nc.dram_tensor("amat", [E, E], DT.float32, kind="ExternalInput").ap()
    bvec = nc.dram_tensor("bvec", [E, 1], DT.float32, kind="ExternalInput").ap()
    out = nc.dram_tensor("out", [TPC, E], DT.float32, kind="ExternalOutput").ap()

    with tile.TileContext(nc) as tc:
        with tc.tile_pool(name="const", bufs=1) as cpool, \
             tc.tile_pool(name="big", bufs=1) as big, \
             tc.tile_pool(name="work", bufs=2) as work, \
             tc.tile_pool(name="ps_chunk", bufs=3, space="PSUM") as ps_chunk, \
             tc.tile_pool(name="ps_tr", bufs=1, space="PSUM") as ps_tr, \
             tc.tile_pool(name="ps_att", bufs=1, space="PSUM") as ps_att, \
             tc.tile_pool(name="dram", bufs=1, space="DRAM") as dram:

            # ---------------- constants ----------------
            ident = cpool.tile([P, P], DT.float32, tag="ident")
            make_identity(nc, ident)

            iotaK = cpool.tile([P, CW], DT.uint32, tag="iotaK")
            nc.gpsimd.iota(iotaK, pattern=[[1, CW]], base=KEYBASE,
                           channel_multiplier=0)

            def const_col(name, val):
                t = cpool.tile([P, 1], DT.uint32, tag=name, name=name)
                nc.gpsimd.iota(t, pattern=[[0, 1]], base=val, channel_multiplier=0)
                return t

            c_shift = const_col("c_shift", KEYSHIFT)
            c_slotmask = const_col("c_slotmask", (1 << KEYSHIFT) - 1)
            c_8 = const_col("c_8", 8)
            c_5 = const_col("c_5", 5)
            c_7 = const_col("c_7", 7)
            c_3 = const_col("c_3", 3)

            # ---------------- emb gather + f32r transpose ----------------
            embT = [big.tile([P, NTOK], DT.float32r, tag=f"embT{kb}",
                             name=f"embT{kb}")
                    for kb in range(2)]
            for m in range(NCORES):
                ti = work.tile([P, 1], DT.int32, tag="ti")
                nc.sync.dma_start(out=ti, in_=tokidx[m * P:(m + 1) * P, :])
                em = work.tile([P, E], DT.float32, tag="em")
                nc.gpsimd.indirect_dma_start(
                    out=em, out_offset=None, in_=table,
                    in_offset=bass.IndirectOffsetOnAxis(ap=ti[:, :], axis=0))
                for kb in range(2):
                    pt = ps_tr.tile([P, P], DT.float32, tag="tr")
                    nc.tensor.transpose(out=pt, in_=em[:, kb * P:(kb + 1) * P],
                                        identity=ident)
                    nc.vector.tensor_copy(embT[kb][:, m * P:(m + 1) * P], pt)

            # own-token embeddings (fp32, for exact rescue dots)
            ti_own = cpool.tile([P, 1], DT.int32, tag="ti_own")
            nc.sync.dma_start(out=ti_own, in_=tok_own)
            emb_own = cpool.tile([P, E], DT.float32, tag="emb_own")
            nc.gpsimd.indirect_dma_start(
                out=emb_own, out_offset=None, in_=table,
                in_offset=bass.IndirectOffsetOnAxis(ap=ti_own[:, :], axis=0))

            # ---------------- small weights ----------------
            a_sb = []
            for kb in range(2):
                t = cpool.tile([P, E], DT.float32, tag=f"a{kb}", name=f"a{kb}")
                nc.sync.dma_start(out=t, in_=amat[kb * P:(kb + 1) * P, :])
                a_sb.append(t)
            a_r = []
            for kb in range(2):
                t = cpool.tile([P, E], DT.float32r, tag=f"ar{kb}", name=f"ar{kb}")
                nc.vector.tensor_copy(t, a_sb[kb])
                a_r.append(t)
            b_sb = []
            for kb in range(2):
                t = cpool.tile([P, 1], DT.float32, tag=f"b{kb}", name=f"b{kb}")
                nc.sync.dma_start(out=t, in_=bvec[kb * P:(kb + 1) * P, :])
                b_sb.append(t)
            b_r = []
            for kb in range(2):
                t = cpool.tile([P, 1], DT.float32r, tag=f"br{kb}", name=f"br{kb}")
                nc.vector.tensor_copy(t, b_sb[kb])
                b_r.append(t)

            # ---------------- streamed tabT strips ----------------
            strip = {}

            def issue_strips(part):
                for j in PARTS[part]:
                    off, w = CHUNKS[j]
                    for kb in range(2):
                        t = big.tile([P, w], DT.float32r, tag=f"tt{kb}_{j % RING}",
                                     name=f"tt{kb}_{j % RING}")
                        nc.sync.dma_start(out=t, in_=tabTr[kb * P:(kb + 1) * P,
                                                           off:off + w])
                        strip[(kb, j)] = t

            issue_strips(0)
            issue_strips(1)

            # ---------------- a2a bounce buffers ----------------
            bounce = [dram.tile([NCORES, TPC, NCP], DT.float32, tag=f"bounce{p}",
                                name=f"bounce{p}")
                      for p in range(4)]
            agg = [dram.tile([NCORES * TPC * NCP, 1], DT.float32, tag=f"agg{p}",
                             name=f"agg{p}")
                   for p in range(4)]
            scd = dram.tile([1, TPC * KP], DT.float32, tag="scd")

            # ---------------- similarity + packed per-chunk top-8 --------
            for part in range(4):
                if part + 2 < 4:
                    issue_strips(part + 2)
                pjs = PARTS[part]
                for m in range(NCORES):
                    if m == 1 and part >= 1:
                        nc.gpsimd.collective_compute(
                            "AllToAll", ALU.bypass,
                            replica_groups=[list(range(NCORES))],
                            ins=[bounce[part - 1][:, :, :].opt()],
                            outs=[agg[part - 1][:, :].opt()],
                        )
                    cv = work.tile([P, NCP], DT.float32, tag="cv")
                    if len(pjs) < 4:
                        nc.vector.memset(cv[:, len(pjs) * 8:], 0.0)
                    for jl, j in enumerate(pjs):
                        off, w = CHUNKS[j]
                        ps = ps_chunk.tile([P, CW], DT.float32, tag="chunk")
                        for kb in range(2):
                            for h in range((w + 511) // 512):
                                hw = min(512, w - h * 512)
                                nc.tensor.matmul(
                                    ps[:, h * 512:h * 512 + hw],
                                    embT[kb][:, m * P:(m + 1) * P],
                                    strip[(kb, j)][:, h * 512:h * 512 + hw],
                                    start=(kb == 0), stop=(kb == 1))
                        keys = work.tile([P, CW], DT.uint32, tag="keys")
                        if j in GP_CHUNKS:
                            # scalar: quantize + exact *2048 shift; gpsimd: +iota
                            q1 = work.tile([P, CW], DT.int32, tag="q1")
                            nc.scalar.activation(q1[:, :w], ps[:, :w],
                                                 ACT.Copy, scale=QSCALE,
                                                 bias=QBIAS)
                            nc.scalar.activation(keys[:, :w].bitcast(DT.int32),
                                                 q1[:, :w], ACT.Copy,
                                                 scale=float(1 << KEYSHIFT))
                            nc.gpsimd.tensor_tensor(keys[:, :w], keys[:, :w],
                                                    iotaK[:, :w], op=ALU.add)
                        else:
                            # quantize logits -> int (scalar engine reads PSUM)
                            nc.scalar.activation(keys[:, :w].bitcast(DT.int32),
                                                 ps[:, :w], ACT.Copy,
                                                 scale=QSCALE, bias=QBIAS)
                            # key = (qi << 11) | slot | 2^30  (one DVE pass)
                            nc.vector.scalar_tensor_tensor(
                                keys[:, :w], keys[:, :w], c_shift[:, :],
                                iotaK[:, :w],
                                op0=ALU.logical_shift_left, op1=ALU.bitwise_or)
                        nc.vector.max(out=cv[:, jl * 8:(jl + 1) * 8],
                                      in_=keys[:, :w].bitcast(DT.float32))
                    nc.sync.dma_start(out=bounce[part][m, :, :], in_=cv)

            nc.gpsimd.collective_compute(
                "AllToAll", ALU.bypass,
                replica_groups=[list(range(NCORES))],
                ins=[bounce[3][:, :, :].opt()],
                outs=[agg[3][:, :].opt()],
            )

            # vals[p, part*256 + c*32 + s] = agg[part][(c, p, s)]
            vals = cpool.tile([P, MERGEW], DT.float32, tag="vals")
            for part in range(4):
                agg_v = agg[part][:, :].rearrange("(c p s) o -> c p (s o)",
                                                  c=NCORES, p=TPC)
                for c in range(NCORES):
                    o = part * NCORES * NCP + c * NCP
                    nc.gpsimd.dma_start(out=vals[:, o:o + NCP], in_=agg_v[c])

            # ---------------- merge: top-20 keys + positions -------------
            wk = cpool.tile([P, KPAD], DT.float32, tag="wk")
            wp = cpool.tile([P, KPAD], DT.uint32, tag="wp")
            vals2 = cpool.tile([P, MERGEW], DT.float32, tag="vals2")
            vals3 = cpool.tile([P, MERGEW], DT.float32, tag="vals3")
            nc.vector.max(out=wk[:, 0:8], in_=vals)
            nc.vector.max_index(out=wp[:, 0:8], in_max=wk[:, 0:8], in_values=vals)
            nc.vector.match_replace(out=vals2, in_to_replace=wk[:, 0:8],
                                    in_values=vals, imm_value=0.0)
            nc.vector.max(out=wk[:, 8:16], in_=vals2)
            nc.vector.max_index(out=wp[:, 8:16], in_max=wk[:, 8:16], in_values=vals2)
            nc.vector.match_replace(out=vals3, in_to_replace=wk[:, 8:16],
                                    in_values=vals2, imm_value=0.0)
            nc.vector.max(out=wk[:, 16:24], in_=vals3)
            nc.vector.max_index(out=wp[:, 16:24], in_max=wk[:, 16:24], in_values=vals3)

            # ---------------- decode global vocab indices ----------------
            # pos = part*256 + c*32 + jl*8 + r ; key = (qi<<11)|slot|2^30
            kp = slice(0, KP)
            slot = cpool.tile([P, KP], DT.uint32, tag="slot")
            nc.vector.tensor_scalar(slot, wk[:, kp].bitcast(DT.uint32),
                                    c_slotmask[:, :], None, op0=ALU.bitwise_and)
            prt = cpool.tile([P, KP], DT.uint32, tag="prt")
            nc.vector.tensor_scalar(prt, wp[:, kp], c_8[:, :], None,
                                    op0=ALU.logical_shift_right)
            csrc = cpool.tile([P, KP], DT.uint32, tag="csrc")
            nc.vector.tensor_scalar(csrc, wp[:, kp], c_5[:, :], None,
                                    op0=ALU.logical_shift_right)
            nc.vector.tensor_scalar(csrc, csrc, c_7[:, :], None,
                                    op0=ALU.bitwise_and)
            jl = cpool.tile([P, KP], DT.uint32, tag="jl")
            nc.vector.tensor_scalar(jl, wp[:, kp], c_3[:, :], None,
                                    op0=ALU.logical_shift_right)
            nc.vector.tensor_scalar(jl, jl, c_3[:, :], None,
                                    op0=ALU.bitwise_and)
            # gidx = csrc*12500 + (3*part + jl)*1024 + slot  (< 2^24: fp-exact)
            gidx = cpool.tile([P, KP], DT.uint32, tag="gidx")
            nc.vector.tensor_scalar(gidx, csrc, float(VS), None, op0=ALU.mult)
            t2 = cpool.tile([P, KP], DT.uint32, tag="t2")
            nc.vector.tensor_scalar(t2, prt, 3.0 * CW, None, op0=ALU.mult)
            nc.vector.tensor_tensor(gidx, gidx, t2, op=ALU.add)
            nc.vector.tensor_scalar(t2, jl, float(CW), None, op0=ALU.mult)
            nc.vector.tensor_tensor(gidx, gidx, t2, op=ALU.add)
            nc.vector.tensor_tensor(gidx, gidx, slot, op=ALU.add)

            # ---------------- rescue: gather h + exact fp32 dots ---------
            hk = [cpool.tile([P, E], DT.float32, tag=f"h{k}", name=f"h{k}")
                  for k in range(KP)]
            gidx_i = gidx[:, :].bitcast(DT.int32)
            for k in range(KP):
                nc.gpsimd.indirect_dma_start(
                    out=hk[k], out_offset=None, in_=table,
                    in_offset=bass.IndirectOffsetOnAxis(ap=gidx_i[:, k:k + 1],
                                                        axis=0))
            d = cpool.tile([P, KPAD], DT.float32, tag="d")
            nc.vector.memset(d[:, KP:], NEG)
            prod = cpool.tile([P, E], DT.float32, tag="prod")
            for k in range(KP):
                nc.vector.scalar_tensor_tensor(
                    prod, hk[k], 1.0, emb_own,
                    op0=ALU.mult, op1=ALU.mult, accum_out=d[:, k:k + 1])

            # 16th largest exact dot -> threshold mask
            t8a = cpool.tile([P, 8], DT.float32, tag="t8a")
            t8b = cpool.tile([P, 8], DT.float32, tag="t8b")
            d2 = cpool.tile([P, KPAD], DT.float32, tag="d2")
            nc.vector.max(out=t8a, in_=d)
            nc.vector.match_replace(out=d2, in_to_replace=t8a, in_values=d,
                                    imm_value=NEG)
            nc.vector.max(out=t8b, in_=d2)
            # maskp = (1[d >= thr16] - 1) * 1e9   (0 for kept, -1e9 for dropped)
            maskp = cpool.tile([P, KP], DT.float32, tag="maskp")
            nc.vector.tensor_scalar(maskp, d[:, :KP], t8b[:, 7:8], None,
                                    op0=ALU.is_ge)
            nc.vector.tensor_scalar(maskp, maskp, -1.0, 1.0e9,
                                    op0=ALU.add, op1=ALU.mult)

            # ---------------- attention over the 20-candidate pool -------
            # hT chunks: n = k*128 + t, grouped 4 k's per 512-wide chunk
            for g in range(NGR):
                hTs = [work.tile([P, 512], DT.float32r, tag=f"hTs{kb}",
                                 name=f"hTs{kb}")
                       for kb in range(2)]
                for kk in range(4):
                    k = g * 4 + kk
                    for kb in range(2):
                        pt = ps_tr.tile([P, P], DT.float32, tag="tr")
                        nc.tensor.transpose(out=pt,
                                            in_=hk[k][:, kb * P:(kb + 1) * P],
                                            identity=ident)
                        nc.vector.tensor_copy(hTs[kb][:, kk * P:(kk + 1) * P], pt)
                tanhTs = [work.tile([P, 512], DT.float32r, tag=f"tanhTs{eb}",
                                    name=f"tanhTs{eb}")
                          for eb in range(2)]
                for eb in range(2):
                    pta = ps_att.tile([P, 512], DT.float32, tag="att")
                    for kb in range(2):
                        nc.tensor.matmul(pta, a_r[kb][:, eb * P:(eb + 1) * P],
                                         hTs[kb], start=(kb == 0), stop=(kb == 1))
                    nc.scalar.activation(tanhTs[eb], pta, ACT.Tanh)
                psc = ps_att.tile([1, 512], DT.float32, tag="att", name="psc")
                for eb in range(2):
                    nc.tensor.matmul(psc, b_r[eb], tanhTs[eb],
                                     start=(eb == 0), stop=(eb == 1))
                scs = work.tile([1, 512], DT.float32, tag="scs")
                nc.vector.tensor_copy(scs, psc)
                nc.sync.dma_start(out=scd[:, g * 512:(g + 1) * 512], in_=scs)

            # scores [t, k] <- scd[k*128 + t]
            sct = cpool.tile([P, KP], DT.float32, tag="sct")
            nc.sync.dma_start(out=sct,
                              in_=scd[:, :].rearrange("o (k t) -> (o t) k", t=TPC))

            # masked softmax over k
            nc.vector.tensor_tensor(sct, sct, maskp, op=ALU.add)
            mx = cpool.tile([P, 1], DT.float32, tag="mx")
            nc.vector.reduce_max(mx, sct, axis=mybir.AxisListType.X)
            negmx = cpool.tile([P, 1], DT.float32, tag="negmx")
            nc.vector.tensor_scalar(negmx, mx, -1.0, None, op0=ALU.mult)
            ex = cpool.tile([P, KP], DT.float32, tag="ex")
            nc.scalar.activation(ex, sct, ACT.Exp, bias=negmx[:, :], scale=1.0)
            sm = cpool.tile([P, 1], DT.float32, tag="sm")
            nc.vector.reduce_sum(sm, ex, axis=mybir.AxisListType.X)
            rc = cpool.tile([P, 1], DT.float32, tag="rc")
            nc.vector.reciprocal(rc, sm)
            att = cpool.tile([P, KP], DT.float32, tag="att_w")
            nc.vector.tensor_scalar(att, ex, rc[:, :], None, op0=ALU.mult)

            # out[t, e] = sum_k att[t,k] * h[t,k,e]
            acc = cpool.tile([P, E], DT.float32, tag="acc")
            nc.vector.memset(acc, 0.0)
            for k in range(KP):
                nc.vector.scalar_tensor_tensor(
                    acc, hk[k], att[:, k:k + 1], acc,
                    op0=ALU.mult, op1=ALU.add)
            nc.sync.dma_start(out=out, in_=acc)

    nc.compile()
    return nc


def get_nc():
    if "v3" not in _BUILD_CACHE:
        _BUILD_CACHE["v3"] = _build()
    return _BUILD_CACHE["v3"]


def kernel(conceptnet_text_vec, table, a, b, topk=16, **_ignored):
    global LAST_RESULTS
    assert int(topk) == TOPK
    tok = np.asarray(conceptnet_text_vec).reshape(NTOK, 1).astype(np.int32)
    table = np.ascontiguousarray(np.asarray(table, dtype=np.float32))
    a = np.ascontiguousarray(np.asarray(a, dtype=np.float32))
    b = np.ascontiguousarray(np.asarray(b, dtype=np.float32)).reshape(E, 1)
    tabT_r = _round12(np.ascontiguousarray(table.T))   # [E, V], f32r-rounded

    nc = get_nc()
    in_maps = []
    for c in range(NCORES):
        in_maps.append({
            "tokidx": tok,
            "tok_own": np.ascontiguousarray(tok[c * TPC:(c + 1) * TPC]),
            "table": table,
            "tabTr": np.ascontiguousarray(tabT_r[:, c * VS:(c + 1) * VS]),
            "amat": a,
            "bvec": b,
        })
    trace = bool(int(os.environ.get("CN_TRACE", "0")))
    res = bass_utils.run_bass_kernel_spmd(nc, in_maps, core_ids=list(range(NCORES)),
                                          trace=trace)
    LAST_RESULTS = res
    outp = np.concatenate([res.results[c]["out"] for c in range(NCORES)], axis=0)
    return outp.reshape(B, L, E)


# revision 13
# speedup vs baseline: 1.1227x; 1.1048x over previous
"""ConceptNet encoder kernel for 8 Trainium2 NeuronCores (Bass/Tile).

Reference computation:
    emb    = table[tok]                      # [1024, 256]
    logits = emb @ table.T                   # [1024, 100000]
    idx    = top16(softmax(logits))          # softmax monotonic -> top16(logits)
    h      = table[idx]                      # [1024, 16, 256]
    e      = tanh(h @ a) @ b                 # [1024, 16]
    out    = softmax(e) @ h                  # [1024, 256]

Distribution: vocab sharded 8 ways. The similarity matmul runs in
float32r (~bf16 speed, ~12-bit mantissa inputs, fp32 accumulate).
Selection packs (quantized value, slot) into one uint32 key per logit
(scalar-engine quantize + one DVE shift-or pass; a few chunks take a
scalar-shift + gpsimd-add path instead to offload the DVE), takes
per-chunk top-8 with a single MAX8 (no FIND_INDEX8, no index plane),
and AllToAll's one key plane per chunk-group (4 pipelined collectives
overlapped with compute). The vocab-shard strips stream through a ring
of SBUF tiles (chunk-group-outer loop), so there is no bulk-load
startup stall. Each core then merges a top-20 candidate pool for its
own 128 tokens, re-scores the pool exactly in fp32 (rescue), and runs
masked-softmax attention over the pool so exactly the true top-16 get
weight.

kernel(**inputs) takes FULL unsharded inputs, returns FULL [4,256,256] output.
Self-contained: hardcodes all shapes; imports only the system concourse repo.
"""
import os
import sys

if "/opt/trn_rl_repo" not in sys.path:
    sys.path.insert(0, "/opt/trn_rl_repo")

import numpy as np

import concourse.bass as bass
import concourse.bacc as bacc
import concourse.mybir as mybir
import concourse.tile as tile
from concourse import bass_utils
from concourse.masks import make_identity

DT = mybir.dt
ALU = mybir.AluOpType
ACT = mybir.ActivationFunctionType

B, L, V, E, TOPK = 4, 256, 100000, 256, 16
NCORES = 8
NTOK = B * L                 # 1024
TPC = NTOK // NCORES         # 128 tokens per core (merge/attention shard)
VS = V // NCORES             # 12500 vocab rows per core
P = 128
NEG = -3.0e38

CW = 1024                    # similarity chunk width (2 PSUM banks)
CHUNKS = []
_off = 0
while _off < VS:
    CHUNKS.append((_off, min(CW, VS - _off)))
    _off += CW
NCHUNK = len(CHUNKS)         # 13 (12x1024 + 212)
PARTS = [[0, 1, 2], [3, 4, 5], [6, 7, 8], [9, 10, 11, 12]]  # j = 3*part + jl
GP_CHUNKS = {1, 4, 7, 11}    # pack pass runs scalar+gpsimd for these chunks
RING = 10                    # streamed tabT strip ring depth (per kb)
NCP = 32                     # candidate slots per (core, part): <=4 chunks x 8
KP = 20                      # rescue pool size per token
KPAD = 24                    # padded pool for max8 rounds
NGR = KP // 4                # attention 512-wide groups
QSCALE = 1536.0              # logit quantizer scale
QBIAS = 3456.0               # makes qi positive (logits in [-2.25, 2])
KEYSHIFT = 11                # slot bits
KEYBASE = 1 << 30            # keeps key bit patterns in normal-float range
MERGEW = 4 * NCORES * NCP    # 1024

_BUILD_CACHE = {}
LAST_RESULTS = None


def _round12(x):
    """Round fp32 to 12 explicit mantissa bits (round half even)."""
    u = np.ascontiguousarray(x, dtype=np.float32).view(np.uint32)
    shift = np.uint32(11)
    mask = np.uint32((1 << 11) - 1)
    half = np.uint32(1 << 10)
    frac = u & mask
    u2 = u & ~mask
    rnd = (frac > half) | ((frac == half) & (((u2 >> shift) & np.uint32(1)) == 1))
    u2 = u2 + (rnd.astype(np.uint32) << shift)
    return u2.view(np.float32)


def _build():
    nc = bacc.Bacc("TRN2", target_bir_lowering=False, debug=False,
                   enable_asserts=True, num_devices=NCORES)

    tokidx = nc.dram_tensor("tokidx", [NTOK, 1], DT.int32, kind="ExternalInput").ap()
    tok_own = nc.dram_tensor("tok_own", [TPC, 1], DT.int32, kind="ExternalInput").ap()
    table = nc.dram_tensor("table", [V, E], DT.float32, kind="ExternalInput").ap()
    tabTr = nc.dram_tensor("tabTr", [E, VS], DT.float32r, kind="ExternalInput").ap()
    amat = nc.dram_tensor("amat", [E, E], DT.float32, kind="ExternalInput").ap()
    bvec = nc.dram_tensor("bvec", [E, 1], DT.float32, kind="ExternalInput").ap()
    out = nc.dram_tensor("out", [TPC, E], DT.float32, kind="ExternalOutput").ap()

    with tile.TileContext(nc) as tc:
        with tc.tile_pool(name="const", bufs=1) as cpool, \
             tc.tile_pool(name="big", bufs=1) as big, \
             tc.tile_pool(name="work", bufs=2) as work, \
             tc.tile_pool(name="ps_chunk", bufs=3, space="PSUM") as ps_chunk, \
             tc.tile_pool(name="ps_tr", bufs=1, space="PSUM") as ps_tr, \
             tc.tile_pool(name="ps_att", bufs=1, space="PSUM") as ps_att, \
             tc.tile_pool(name="dram", bufs=1, space="DRAM") as dram:

            # ---------------- constants ----------------
            ident = cpool.tile([P, P], DT.float32, tag="ident")
            make_identity(nc, ident)

            iotaK = cpool.tile([P, CW], DT.uint32, tag="iotaK")
            nc.gpsimd.iota(iotaK, pattern=[[1, CW]], base=KEYBASE,
                           channel_multiplier=0)

            def const_col(name, val):
                t = cpool.tile([P, 1], DT.uint32, tag=name, name=name)
                nc.gpsimd.iota(t, pattern=[[0, 1]], base=val, channel_multiplier=0)
                return t

            c_shift = const_col("c_shift", KEYSHIFT)
            c_slotmask = const_col("c_slotmask", (1 << KEYSHIFT) - 1)
            c_8 = const_col("c_8", 8)
            c_5 = const_col("c_5", 5)
            c_7 = const_col("c_7", 7)
            c_3 = const_col("c_3", 3)

            # ---------------- emb gather + f32r transpose ----------------
            embT = [big.tile([P, NTOK], DT.float32r, tag=f"embT{kb}",
                             name=f"embT{kb}")
                    for kb in range(2)]
            for m in range(NCORES):
                ti = work.tile([P, 1], DT.int32, tag="ti")
                nc.sync.dma_start(out=ti, in_=tokidx[m * P:(m + 1) * P, :])
                em = work.tile([P, E], DT.float32, tag="em")
                nc.gpsimd.indirect_dma_start(
                    out=em, out_offset=None, in_=table,
                    in_offset=bass.IndirectOffsetOnAxis(ap=ti[:, :], axis=0))
                for kb in range(2):
                    pt = ps_tr.tile([P, P], DT.float32, tag="tr")
                    nc.tensor.transpose(out=pt, in_=em[:, kb * P:(kb + 1) * P],
                                        identity=ident)
                    nc.vector.tensor_copy(embT[kb][:, m * P:(m + 1) * P], pt)

            # own-token embeddings (fp32, for exact rescue dots)
            ti_own = cpool.tile([P, 1], DT.int32, tag="ti_own")
            nc.sync.dma_start(out=ti_own, in_=tok_own)
            emb_own = cpool.tile([P, E], DT.float32, tag="emb_own")
            nc.gpsimd.indirect_dma_start(
                out=emb_own, out_offset=None, in_=table,
                in_offset=bass.IndirectOffsetOnAxis(ap=ti_own[:, :], axis=0))

            # ---------------- small weights ----------------
            a_sb = []
            for kb in range(2):
                t = cpool.tile([P, E], DT.float32, tag=f"a{kb}", name=f"a{kb}")
                nc.sync.dma_start(out=t, in_=amat[kb * P:(kb + 1) * P, :])
                a_sb.append(t)
            a_r = []
            for kb in range(2):
                t = cpool.tile([P, E], DT.float32r, tag=f"ar{kb}", name=f"ar{kb}")
                nc.vector.tensor_copy(t, a_sb[kb])
                a_r.append(t)
            b_sb = []
            for kb in range(2):
                t = cpool.tile([P, 1], DT.float32, tag=f"b{kb}", name=f"b{kb}")
                nc.sync.dma_start(out=t, in_=bvec[kb * P:(kb + 1) * P, :])
                b_sb.append(t)
            b_r = []
            for kb in range(2):
                t = cpool.tile([P, 1], DT.float32r, tag=f"br{kb}", name=f"br{kb}")
                nc.vector.tensor_copy(t, b_sb[kb])
                b_r.append(t)

            # ---------------- streamed tabT strips ----------------
            strip = {}

            def issue_strips(part):
                for j in PARTS[part]:
                    off, w = CHUNKS[j]
                    for kb in range(2):
                        t = big.tile([P, w], DT.float32r, tag=f"tt{kb}_{j % RING}",
                                     name=f"tt{kb}_{j % RING}")
                        nc.sync.dma_start(out=t, in_=tabTr[kb * P:(kb + 1) * P,
                                                           off:off + w])
                        strip[(kb, j)] = t

            issue_strips(0)
            issue_strips(1)

            # ---------------- a2a bounce buffers ----------------
            bounce = [dram.tile([NCORES, TPC, NCP], DT.float32, tag=f"bounce{p}",
                                name=f"bounce{p}")
                      for p in range(4)]
            agg = [dram.tile([NCORES * TPC * NCP, 1], DT.float32, tag=f"agg{p}",
                             name=f"agg{p}")
                   for p in range(4)]
            scd = dram.tile([1, TPC * KP], DT.float32, tag="scd")

            # ---------------- similarity + packed per-chunk top-8 --------
            for part in range(4):
                if part + 2 < 4:
                    issue_strips(part + 2)
                pjs = PARTS[part]
                for m in range(NCORES):
                    if m == 1 and part >= 1:
                        nc.gpsimd.collective_compute(
                            "AllToAll", ALU.bypass,
                            replica_groups=[list(range(NCORES))],
                            ins=[bounce[part - 1][:, :, :].opt()],
                            outs=[agg[part - 1][:, :].opt()],
                        )
                    cv = work.tile([P, NCP], DT.float32, tag="cv")
                    if len(pjs) < 4:
                        nc.vector.memset(cv[:, len(pjs) * 8:], 0.0)
                    for jl, j in enumerate(pjs):
                        off, w = CHUNKS[j]
                        ps = ps_chunk.tile([P, CW], DT.float32, tag="chunk")
                        for kb in range(2):
                            for h in range((w + 511) // 512):
                                hw = min(512, w - h * 512)
                                nc.tensor.matmul(
                                    ps[:, h * 512:h * 512 + hw],
                                    embT[kb][:, m * P:(m + 1) * P],
                                    strip[(kb, j)][:, h * 512:h * 512 + hw],
                                    start=(kb == 0), stop=(kb == 1))
                        keys = work.tile([P, CW], DT.uint32, tag="keys", bufs=4)
                        if j in GP_CHUNKS:
                            # scalar: quantize + exact *2048 shift; gpsimd: +iota
                            q1 = work.tile([P, CW], DT.int32, tag="q1", bufs=3)
                            nc.scalar.activation(q1[:, :w], ps[:, :w],
                                                 ACT.Copy, scale=QSCALE,
                                                 bias=QBIAS)
                            nc.scalar.activation(keys[:, :w].bitcast(DT.int32),
                                                 q1[:, :w], ACT.Copy,
                                                 scale=float(1 << KEYSHIFT))
                            nc.gpsimd.tensor_tensor(keys[:, :w], keys[:, :w],
                                                    iotaK[:, :w], op=ALU.add)
                        else:
                            # quantize logits -> int (scalar engine reads PSUM)
                            nc.scalar.activation(keys[:, :w].bitcast(DT.int32),
                                                 ps[:, :w], ACT.Copy,
                                                 scale=QSCALE, bias=QBIAS)
                            # key = (qi << 11) | slot | 2^30  (one DVE pass)
                            nc.vector.scalar_tensor_tensor(
                                keys[:, :w], keys[:, :w], c_shift[:, :],
                                iotaK[:, :w],
                                op0=ALU.logical_shift_left, op1=ALU.bitwise_or)
                        nc.vector.max(out=cv[:, jl * 8:(jl + 1) * 8],
                                      in_=keys[:, :w].bitcast(DT.float32))
                    nc.sync.dma_start(out=bounce[part][m, :, :], in_=cv)

            nc.gpsimd.collective_compute(
                "AllToAll", ALU.bypass,
                replica_groups=[list(range(NCORES))],
                ins=[bounce[3][:, :, :].opt()],
                outs=[agg[3][:, :].opt()],
            )

            # vals[p, part*256 + c*32 + s] = agg[part][(c, p, s)]
            vals = cpool.tile([P, MERGEW], DT.float32, tag="vals")
            for part in range(4):
                agg_v = agg[part][:, :].rearrange("(c p s) o -> c p (s o)",
                                                  c=NCORES, p=TPC)
                for c in range(NCORES):
                    o = part * NCORES * NCP + c * NCP
                    nc.gpsimd.dma_start(out=vals[:, o:o + NCP], in_=agg_v[c])

            # ---------------- merge: top-20 keys + positions -------------
            wk = cpool.tile([P, KPAD], DT.float32, tag="wk")
            wp = cpool.tile([P, KPAD], DT.uint32, tag="wp")
            vals2 = cpool.tile([P, MERGEW], DT.float32, tag="vals2")
            vals3 = cpool.tile([P, MERGEW], DT.float32, tag="vals3")
            nc.vector.max(out=wk[:, 0:8], in_=vals)
            nc.vector.max_index(out=wp[:, 0:8], in_max=wk[:, 0:8], in_values=vals)
            nc.vector.match_replace(out=vals2, in_to_replace=wk[:, 0:8],
                                    in_values=vals, imm_value=0.0)
            nc.vector.max(out=wk[:, 8:16], in_=vals2)
            nc.vector.max_index(out=wp[:, 8:16], in_max=wk[:, 8:16], in_values=vals2)
            nc.vector.match_replace(out=vals3, in_to_replace=wk[:, 8:16],
                                    in_values=vals2, imm_value=0.0)
            nc.vector.max(out=wk[:, 16:24], in_=vals3)
            nc.vector.max_index(out=wp[:, 16:24], in_max=wk[:, 16:24], in_values=vals3)

            # ---------------- decode global vocab indices ----------------
            # pos = part*256 + c*32 + jl*8 + r ; key = (qi<<11)|slot|2^30
            kp = slice(0, KP)
            slot = cpool.tile([P, KP], DT.uint32, tag="slot")
            nc.vector.tensor_scalar(slot, wk[:, kp].bitcast(DT.uint32),
                                    c_slotmask[:, :], None, op0=ALU.bitwise_and)
            prt = cpool.tile([P, KP], DT.uint32, tag="prt")
            nc.vector.tensor_scalar(prt, wp[:, kp], c_8[:, :], None,
                                    op0=ALU.logical_shift_right)
            csrc = cpool.tile([P, KP], DT.uint32, tag="csrc")
            nc.vector.tensor_scalar(csrc, wp[:, kp], c_5[:, :], None,
                                    op0=ALU.logical_shift_right)
            nc.vector.tensor_scalar(csrc, csrc, c_7[:, :], None,
                                    op0=ALU.bitwise_and)
            jl = cpool.tile([P, KP], DT.uint32, tag="jl")
            nc.vector.tensor_scalar(jl, wp[:, kp], c_3[:, :], None,
                                    op0=ALU.logical_shift_right)
            nc.vector.tensor_scalar(jl, jl, c_3[:, :], None,
                                    op0=ALU.bitwise_and)
            # gidx = csrc*12500 + (3*part + jl)*1024 + slot  (< 2^24: fp-exact)
            gidx = cpool.tile([P, KP], DT.uint32, tag="gidx")
            nc.vector.tensor_scalar(gidx, csrc, float(VS), None, op0=ALU.mult)
            t2 = cpool.tile([P, KP], DT.uint32, tag="t2")
            nc.vector.tensor_scalar(t2, prt, 3.0 * CW, None, op0=ALU.mult)
            nc.vector.tensor_tensor(gidx, gidx, t2, op=ALU.add)
            nc.vector.tensor_scalar(t2, jl, float(CW), None, op0=ALU.mult)
            nc.vector.tensor_tensor(gidx, gidx, t2, op=ALU.add)
            nc.vector.tensor_tensor(gidx, gidx, slot, op=ALU.add)

            # ---------------- rescue: gather h + exact fp32 dots ---------
            hk = [cpool.tile([P, E], DT.float32, tag=f"h{k}", name=f"h{k}")
                  for k in range(KP)]
            gidx_i = gidx[:, :].bitcast(DT.int32)
            for k in range(KP):
                nc.gpsimd.indirect_dma_start(
                    out=hk[k], out_offset=None, in_=table,
                    in_offset=bass.IndirectOffsetOnAxis(ap=gidx_i[:, k:k + 1],
                                                        axis=0))
            d = cpool.tile([P, KPAD], DT.float32, tag="d")
            nc.vector.memset(d[:, KP:], NEG)
            prod = cpool.tile([P, E], DT.float32, tag="prod")
            for k in range(KP):
                nc.vector.scalar_tensor_tensor(
                    prod, hk[k], 1.0, emb_own,
                    op0=ALU.mult, op1=ALU.mult, accum_out=d[:, k:k + 1])

            # 16th largest exact dot -> threshold mask
            t8a = cpool.tile([P, 8], DT.float32, tag="t8a")
            t8b = cpool.tile([P, 8], DT.float32, tag="t8b")
            d2 = cpool.tile([P, KPAD], DT.float32, tag="d2")
            nc.vector.max(out=t8a, in_=d)
            nc.vector.match_replace(out=d2, in_to_replace=t8a, in_values=d,
                                    imm_value=NEG)
            nc.vector.max(out=t8b, in_=d2)
            # maskp = (1[d >= thr16] - 1) * 1e9   (0 for kept, -1e9 for dropped)
            maskp = cpool.tile([P, KP], DT.float32, tag="maskp")
            nc.vector.tensor_scalar(maskp, d[:, :KP], t8b[:, 7:8], None,
                                    op0=ALU.is_ge)
            nc.vector.tensor_scalar(maskp, maskp, -1.0, 1.0e9,
                                    op0=ALU.add, op1=ALU.mult)

            # ---------------- attention over the 20-candidate pool -------
            # hT chunks: n = k*128 + t, grouped 4 k's per 512-wide chunk
            for g in range(NGR):
                hTs = [work.tile([P, 512], DT.float32r, tag=f"hTs{kb}",
                                 name=f"hTs{kb}")
                       for kb in range(2)]
                for kk in range(4):
                    k = g * 4 + kk
                    for kb in range(2):
                        pt = ps_tr.tile([P, P], DT.float32, tag="tr")
                        nc.tensor.transpose(out=pt,
                                            in_=hk[k][:, kb * P:(kb + 1) * P],
                                            identity=ident)
                        nc.vector.tensor_copy(hTs[kb][:, kk * P:(kk + 1) * P], pt)
                tanhTs = [work.tile([P, 512], DT.float32r, tag=f"tanhTs{eb}",
                                    name=f"tanhTs{eb}")
                          for eb in range(2)]
                for eb in range(2):
                    pta = ps_att.tile([P, 512], DT.float32, tag="att")
                    for kb in range(2):
                        nc.tensor.matmul(pta, a_r[kb][:, eb * P:(eb + 1) * P],
                                         hTs[kb], start=(kb == 0), stop=(kb == 1))
                    nc.scalar.activation(tanhTs[eb], pta, ACT.Tanh)
                psc = ps_att.tile([1, 512], DT.float32, tag="att", name="psc")
                for eb in range(2):
                    nc.tensor.matmul(psc, b_r[eb], tanhTs[eb],
                                     start=(eb == 0), stop=(eb == 1))
                scs = work.tile([1, 512], DT.float32, tag="scs")
                nc.vector.tensor_copy(scs, psc)
                nc.sync.dma_start(out=scd[:, g * 512:(g + 1) * 512], in_=scs)

            # scores [t, k] <- scd[k*128 + t]
            sct = cpool.tile([P, KP], DT.float32, tag="sct")
            nc.sync.dma_start(out=sct,
                              in_=scd[:, :].rearrange("o (k t) -> (o t) k", t=TPC))

            # masked softmax over k
            nc.vector.tensor_tensor(sct, sct, maskp, op=ALU.add)
            mx = cpool.tile([P, 1], DT.float32, tag="mx")
            nc.vector.reduce_max(mx, sct, axis=mybir.AxisListType.X)
            negmx = cpool.tile([P, 1], DT.float32, tag="negmx")
            nc.vector.tensor_scalar(negmx, mx, -1.0, None, op0=ALU.mult)
            ex = cpool.tile([P, KP], DT.float32, tag="ex")
            nc.scalar.activation(ex, sct, ACT.Exp, bias=negmx[:, :], scale=1.0)
            sm = cpool.tile([P, 1], DT.float32, tag="sm")
            nc.vector.reduce_sum(sm, ex, axis=mybir.AxisListType.X)
            rc = cpool.tile([P, 1], DT.float32, tag="rc")
            nc.vector.reciprocal(rc, sm)
            att = cpool.tile([P, KP], DT.float32, tag="att_w")
            nc.vector.tensor_scalar(att, ex, rc[:, :], None, op0=ALU.mult)

            # out[t, e] = sum_k att[t,k] * h[t,k,e]
            acc = cpool.tile([P, E], DT.float32, tag="acc")
            nc.vector.memset(acc, 0.0)
            for k in range(KP):
                nc.vector.scalar_tensor_tensor(
                    acc, hk[k], att[:, k:k + 1], acc,
                    op0=ALU.mult, op1=ALU.add)
            nc.sync.dma_start(out=out, in_=acc)

    nc.compile()
    return nc


def get_nc():
    if "v3" not in _BUILD_CACHE:
        _BUILD_CACHE["v3"] = _build()
    return _BUILD_CACHE["v3"]


def kernel(conceptnet_text_vec, table, a, b, topk=16, **_ignored):
    global LAST_RESULTS
    assert int(topk) == TOPK
    tok = np.asarray(conceptnet_text_vec).reshape(NTOK, 1).astype(np.int32)
    table = np.ascontiguousarray(np.asarray(table, dtype=np.float32))
    a = np.ascontiguousarray(np.asarray(a, dtype=np.float32))
    b = np.ascontiguousarray(np.asarray(b, dtype=np.float32)).reshape(E, 1)
    tabT_r = _round12(np.ascontiguousarray(table.T))   # [E, V], f32r-rounded

    nc = get_nc()
    in_maps = []
    for c in range(NCORES):
        in_maps.append({
            "tokidx": tok,
            "tok_own": np.ascontiguousarray(tok[c * TPC:(c + 1) * TPC]),
            "table": table,
            "tabTr": np.ascontiguousarray(tabT_r[:, c * VS:(c + 1) * VS]),
            "amat": a,
            "bvec": b,
        })
    trace = bool(int(os.environ.get("CN_TRACE", "0")))
    res = bass_utils.run_bass_kernel_spmd(nc, in_maps, core_ids=list(range(NCORES)),
                                          trace=trace)
    LAST_RESULTS = res
    outp = np.concatenate([res.results[c]["out"] for c in range(NCORES)], axis=0)
    return outp.reshape(B, L, E)


# revision 14
# speedup vs baseline: 1.1555x; 1.0292x over previous
"""ConceptNet encoder kernel for 8 Trainium2 NeuronCores (Bass/Tile).

Reference computation:
    emb    = table[tok]                      # [1024, 256]
    logits = emb @ table.T                   # [1024, 100000]
    idx    = top16(softmax(logits))          # softmax monotonic -> top16(logits)
    h      = table[idx]                      # [1024, 16, 256]
    e      = tanh(h @ a) @ b                 # [1024, 16]
    out    = softmax(e) @ h                  # [1024, 256]

Distribution: vocab sharded 8 ways. The similarity matmul runs in
float32r (~bf16 speed, ~12-bit mantissa inputs, fp32 accumulate).
Selection packs (quantized value, slot) into one uint32 key per logit
(scalar-engine quantize + one DVE shift-or pass; a few chunks take a
scalar-shift + gpsimd-add path instead to offload the DVE), takes
per-chunk top-8 with a single MAX8 (no FIND_INDEX8, no index plane),
and AllToAll's one key plane per chunk-group (4 pipelined collectives
overlapped with compute). The vocab-shard strips stream through a ring
of SBUF tiles (chunk-group-outer loop), so there is no bulk-load
startup stall. Each core then merges a top-20 candidate pool for its
own 128 tokens, re-scores the pool exactly in fp32 (rescue), and runs
masked-softmax attention over the pool so exactly the true top-16 get
weight.

kernel(**inputs) takes FULL unsharded inputs, returns FULL [4,256,256] output.
Self-contained: hardcodes all shapes; imports only the system concourse repo.
"""
import os
import sys

if "/opt/trn_rl_repo" not in sys.path:
    sys.path.insert(0, "/opt/trn_rl_repo")

import numpy as np

import concourse.bass as bass
import concourse.bacc as bacc
import concourse.mybir as mybir
import concourse.tile as tile
from concourse import bass_utils
from concourse.masks import make_identity

DT = mybir.dt
ALU = mybir.AluOpType
ACT = mybir.ActivationFunctionType

B, L, V, E, TOPK = 4, 256, 100000, 256, 16
NCORES = 8
NTOK = B * L                 # 1024
TPC = NTOK // NCORES         # 128 tokens per core (merge/attention shard)
VS = V // NCORES             # 12500 vocab rows per core
P = 128
NEG = -3.0e38

CW = 1024                    # similarity chunk width (2 PSUM banks)
CHUNKS = []
_off = 0
while _off < VS:
    CHUNKS.append((_off, min(CW, VS - _off)))
    _off += CW
NCHUNK = len(CHUNKS)         # 13 (12x1024 + 212)
PARTS = [[0, 1, 2], [3, 4, 5], [6, 7, 8], [9, 10, 11, 12]]  # j = 3*part + jl
GP_CHUNKS = {1, 4, 7, 11}    # pack pass runs scalar+gpsimd for these chunks
RING = 10                    # streamed tabT strip ring depth (per kb)
NCP = 32                     # candidate slots per (core, part): <=4 chunks x 8
KP = 20                      # rescue pool size per token
KPAD = 24                    # padded pool for max8 rounds
NGR = KP // 4                # attention 512-wide groups
QSCALE = 1536.0              # logit quantizer scale
QBIAS = 3456.0               # makes qi positive (logits in [-2.25, 2])
KEYSHIFT = 11                # slot bits
KEYBASE = 1 << 30            # keeps key bit patterns in normal-float range
MERGEW = 4 * NCORES * NCP    # 1024

_BUILD_CACHE = {}
LAST_RESULTS = None


def _round12(x):
    """Round fp32 to 12 explicit mantissa bits (round half even)."""
    u = np.ascontiguousarray(x, dtype=np.float32).view(np.uint32)
    shift = np.uint32(11)
    mask = np.uint32((1 << 11) - 1)
    half = np.uint32(1 << 10)
    frac = u & mask
    u2 = u & ~mask
    rnd = (frac > half) | ((frac == half) & (((u2 >> shift) & np.uint32(1)) == 1))
    u2 = u2 + (rnd.astype(np.uint32) << shift)
    return u2.view(np.float32)


def _build():
    nc = bacc.Bacc("TRN2", target_bir_lowering=False, debug=False,
                   enable_asserts=True, num_devices=NCORES)

    tokidx = nc.dram_tensor("tokidx", [NTOK, 1], DT.int32, kind="ExternalInput").ap()
    tok_own = nc.dram_tensor("tok_own", [TPC, 1], DT.int32, kind="ExternalInput").ap()
    table = nc.dram_tensor("table", [V, E], DT.float32, kind="ExternalInput").ap()
    tabTr = nc.dram_tensor("tabTr", [E, VS], DT.float32r, kind="ExternalInput").ap()
    amat = nc.dram_tensor("amat", [E, E], DT.float32, kind="ExternalInput").ap()
    bvec = nc.dram_tensor("bvec", [E, 1], DT.float32, kind="ExternalInput").ap()
    out = nc.dram_tensor("out", [TPC, E], DT.float32, kind="ExternalOutput").ap()

    with tile.TileContext(nc) as tc:
        with tc.tile_pool(name="const", bufs=1) as cpool, \
             tc.tile_pool(name="big", bufs=1) as big, \
             tc.tile_pool(name="work", bufs=2) as work, \
             tc.tile_pool(name="ps_chunk", bufs=3, space="PSUM") as ps_chunk, \
             tc.tile_pool(name="ps_tr", bufs=2, space="PSUM") as ps_tr, \
             tc.tile_pool(name="dram", bufs=1, space="DRAM") as dram:

            # ---------------- constants ----------------
            ident = cpool.tile([P, P], DT.float32, tag="ident")
            make_identity(nc, ident)

            iotaK = cpool.tile([P, CW], DT.uint32, tag="iotaK")
            nc.gpsimd.iota(iotaK, pattern=[[1, CW]], base=KEYBASE,
                           channel_multiplier=0)

            def const_col(name, val):
                t = cpool.tile([P, 1], DT.uint32, tag=name, name=name)
                nc.gpsimd.iota(t, pattern=[[0, 1]], base=val, channel_multiplier=0)
                return t

            c_shift = const_col("c_shift", KEYSHIFT)
            c_slotmask = const_col("c_slotmask", (1 << KEYSHIFT) - 1)
            c_8 = const_col("c_8", 8)
            c_5 = const_col("c_5", 5)
            c_7 = const_col("c_7", 7)
            c_3 = const_col("c_3", 3)

            # ---------------- emb gather + f32r transpose ----------------
            embT = [[big.tile([P, P], DT.float32r, tag=f"embT{kb}_{m}",
                              name=f"embT{kb}_{m}")
                     for m in range(NCORES)] for kb in range(2)]
            for m in range(NCORES):
                ti = work.tile([P, 1], DT.int32, tag="ti")
                nc.sync.dma_start(out=ti, in_=tokidx[m * P:(m + 1) * P, :])
                em = work.tile([P, E], DT.float32, tag="em")
                nc.gpsimd.indirect_dma_start(
                    out=em, out_offset=None, in_=table,
                    in_offset=bass.IndirectOffsetOnAxis(ap=ti[:, :], axis=0))
                for kb in range(2):
                    pt = ps_tr.tile([P, P], DT.float32, tag="tr")
                    nc.tensor.transpose(out=pt, in_=em[:, kb * P:(kb + 1) * P],
                                        identity=ident)
                    nc.vector.tensor_copy(embT[kb][m], pt)

            # own-token embeddings (fp32, for exact rescue dots)
            ti_own = cpool.tile([P, 1], DT.int32, tag="ti_own")
            nc.sync.dma_start(out=ti_own, in_=tok_own)
            emb_own = cpool.tile([P, E], DT.float32, tag="emb_own")
            nc.gpsimd.indirect_dma_start(
                out=emb_own, out_offset=None, in_=table,
                in_offset=bass.IndirectOffsetOnAxis(ap=ti_own[:, :], axis=0))

            # ---------------- small weights ----------------
            a_sb = []
            for kb in range(2):
                t = cpool.tile([P, E], DT.float32, tag=f"a{kb}", name=f"a{kb}")
                nc.sync.dma_start(out=t, in_=amat[kb * P:(kb + 1) * P, :])
                a_sb.append(t)
            a_r = []
            for kb in range(2):
                t = cpool.tile([P, E], DT.float32r, tag=f"ar{kb}", name=f"ar{kb}")
                nc.vector.tensor_copy(t, a_sb[kb])
                a_r.append(t)
            b_sb = []
            for kb in range(2):
                t = cpool.tile([P, 1], DT.float32, tag=f"b{kb}", name=f"b{kb}")
                nc.sync.dma_start(out=t, in_=bvec[kb * P:(kb + 1) * P, :])
                b_sb.append(t)
            b_r = []
            for kb in range(2):
                t = cpool.tile([P, 1], DT.float32r, tag=f"br{kb}", name=f"br{kb}")
                nc.vector.tensor_copy(t, b_sb[kb])
                b_r.append(t)

            # ---------------- streamed tabT strips ----------------
            strip = {}

            def issue_strips(part):
                for j in PARTS[part]:
                    off, w = CHUNKS[j]
                    for kb in range(2):
                        t = big.tile([P, w], DT.float32r, tag=f"tt{kb}_{j % RING}",
                                     name=f"tt{kb}_{j % RING}")
                        nc.sync.dma_start(out=t, in_=tabTr[kb * P:(kb + 1) * P,
                                                           off:off + w])
                        strip[(kb, j)] = t

            issue_strips(0)
            issue_strips(1)

            vals = cpool.tile([P, MERGEW], DT.float32, tag="vals")

            def load_vals(part):
                # vals[p, part*256 + c*32 + s] = agg[part][(c, p, s)]
                agg_v = agg[part][:, :].rearrange("(c p s) o -> c p (s o)",
                                                  c=NCORES, p=TPC)
                for c in range(NCORES):
                    o = part * NCORES * NCP + c * NCP
                    nc.gpsimd.dma_start(out=vals[:, o:o + NCP], in_=agg_v[c])

            # ---------------- a2a bounce buffers ----------------
            bounce = [dram.tile([NCORES, TPC, NCP], DT.float32, tag=f"bounce{p}",
                                name=f"bounce{p}")
                      for p in range(4)]
            agg = [dram.tile([NCORES * TPC * NCP, 1], DT.float32, tag=f"agg{p}",
                             name=f"agg{p}")
                   for p in range(4)]
            scd = dram.tile([1, TPC * KP], DT.float32, tag="scd")

            # ---------------- similarity + packed per-chunk top-8 --------
            for part in range(4):
                if part + 2 < 4:
                    issue_strips(part + 2)
                pjs = PARTS[part]
                for m in range(NCORES):
                    if m == 1 and part >= 1:
                        nc.gpsimd.collective_compute(
                            "AllToAll", ALU.bypass,
                            replica_groups=[list(range(NCORES))],
                            ins=[bounce[part - 1][:, :, :].opt()],
                            outs=[agg[part - 1][:, :].opt()],
                        )
                    if m == 5 and part >= 1:
                        load_vals(part - 1)
                    cv = work.tile([P, NCP], DT.float32, tag="cv")
                    if len(pjs) < 4:
                        nc.vector.memset(cv[:, len(pjs) * 8:], 0.0)
                    for jl, j in enumerate(pjs):
                        off, w = CHUNKS[j]
                        ps = ps_chunk.tile([P, CW], DT.float32, tag="chunk")
                        for kb in range(2):
                            for h in range((w + 511) // 512):
                                hw = min(512, w - h * 512)
                                nc.tensor.matmul(
                                    ps[:, h * 512:h * 512 + hw],
                                    embT[kb][m],
                                    strip[(kb, j)][:, h * 512:h * 512 + hw],
                                    start=(kb == 0), stop=(kb == 1))
                        keys = work.tile([P, CW], DT.uint32, tag="keys", bufs=4)
                        if j in GP_CHUNKS:
                            # scalar: quantize + exact *2048 shift; gpsimd: +iota
                            q1 = work.tile([P, CW], DT.int32, tag="q1", bufs=3)
                            nc.scalar.activation(q1[:, :w], ps[:, :w],
                                                 ACT.Copy, scale=QSCALE,
                                                 bias=QBIAS)
                            nc.scalar.activation(keys[:, :w].bitcast(DT.int32),
                                                 q1[:, :w], ACT.Copy,
                                                 scale=float(1 << KEYSHIFT))
                            nc.gpsimd.tensor_tensor(keys[:, :w], keys[:, :w],
                                                    iotaK[:, :w], op=ALU.add)
                        else:
                            # quantize logits -> int (scalar engine reads PSUM)
                            nc.scalar.activation(keys[:, :w].bitcast(DT.int32),
                                                 ps[:, :w], ACT.Copy,
                                                 scale=QSCALE, bias=QBIAS)
                            # key = (qi << 11) | slot | 2^30  (one DVE pass)
                            nc.vector.scalar_tensor_tensor(
                                keys[:, :w], keys[:, :w], c_shift[:, :],
                                iotaK[:, :w],
                                op0=ALU.logical_shift_left, op1=ALU.bitwise_or)
                        nc.vector.max(out=cv[:, jl * 8:(jl + 1) * 8],
                                      in_=keys[:, :w].bitcast(DT.float32))
                    nc.sync.dma_start(out=bounce[part][m, :, :], in_=cv)

            nc.gpsimd.collective_compute(
                "AllToAll", ALU.bypass,
                replica_groups=[list(range(NCORES))],
                ins=[bounce[3][:, :, :].opt()],
                outs=[agg[3][:, :].opt()],
            )

            for part in (3,):
                load_vals(part)

            # ---------------- merge: top-20 keys + positions -------------
            wk = cpool.tile([P, KPAD], DT.float32, tag="wk")
            wp = cpool.tile([P, KPAD], DT.uint32, tag="wp")
            vals2 = cpool.tile([P, MERGEW], DT.float32, tag="vals2")
            vals3 = cpool.tile([P, MERGEW], DT.float32, tag="vals3")
            nc.vector.max(out=wk[:, 0:8], in_=vals)
            nc.vector.max_index(out=wp[:, 0:8], in_max=wk[:, 0:8], in_values=vals)
            nc.vector.match_replace(out=vals2, in_to_replace=wk[:, 0:8],
                                    in_values=vals, imm_value=0.0)
            nc.vector.max(out=wk[:, 8:16], in_=vals2)
            nc.vector.max_index(out=wp[:, 8:16], in_max=wk[:, 8:16], in_values=vals2)
            nc.vector.match_replace(out=vals3, in_to_replace=wk[:, 8:16],
                                    in_values=vals2, imm_value=0.0)
            nc.vector.max(out=wk[:, 16:24], in_=vals3)
            nc.vector.max_index(out=wp[:, 16:24], in_max=wk[:, 16:24], in_values=vals3)

            # ---------------- decode global vocab indices ----------------
            # pos = part*256 + c*32 + jl*8 + r ; key = (qi<<11)|slot|2^30
            kp = slice(0, KP)
            slot = cpool.tile([P, KP], DT.uint32, tag="slot")
            nc.vector.tensor_scalar(slot, wk[:, kp].bitcast(DT.uint32),
                                    c_slotmask[:, :], None, op0=ALU.bitwise_and)
            prt = cpool.tile([P, KP], DT.uint32, tag="prt")
            nc.vector.tensor_scalar(prt, wp[:, kp], c_8[:, :], None,
                                    op0=ALU.logical_shift_right)
            csrc = cpool.tile([P, KP], DT.uint32, tag="csrc")
            nc.vector.tensor_scalar(csrc, wp[:, kp], c_5[:, :], None,
                                    op0=ALU.logical_shift_right)
            nc.vector.tensor_scalar(csrc, csrc, c_7[:, :], None,
                                    op0=ALU.bitwise_and)
            jl = cpool.tile([P, KP], DT.uint32, tag="jl")
            nc.vector.tensor_scalar(jl, wp[:, kp], c_3[:, :], None,
                                    op0=ALU.logical_shift_right)
            nc.vector.tensor_scalar(jl, jl, c_3[:, :], None,
                                    op0=ALU.bitwise_and)
            # gidx = csrc*12500 + (3*part + jl)*1024 + slot  (< 2^24: fp-exact)
            gidx = cpool.tile([P, KP], DT.uint32, tag="gidx")
            nc.vector.tensor_scalar(gidx, csrc, float(VS), None, op0=ALU.mult)
            t2 = cpool.tile([P, KP], DT.uint32, tag="t2")
            nc.vector.tensor_scalar(t2, prt, 3.0 * CW, None, op0=ALU.mult)
            nc.vector.tensor_tensor(gidx, gidx, t2, op=ALU.add)
            nc.vector.tensor_scalar(t2, jl, float(CW), None, op0=ALU.mult)
            nc.vector.tensor_tensor(gidx, gidx, t2, op=ALU.add)
            nc.vector.tensor_tensor(gidx, gidx, slot, op=ALU.add)

            # ---------------- rescue: gather h + exact fp32 dots ---------
            hk = [cpool.tile([P, E], DT.float32, tag=f"h{k}", name=f"h{k}")
                  for k in range(KP)]
            gidx_i = gidx[:, :].bitcast(DT.int32)
            for k in range(KP):
                nc.gpsimd.indirect_dma_start(
                    out=hk[k], out_offset=None, in_=table,
                    in_offset=bass.IndirectOffsetOnAxis(ap=gidx_i[:, k:k + 1],
                                                        axis=0))
            d = cpool.tile([P, KPAD], DT.float32, tag="d")
            nc.vector.memset(d[:, KP:], NEG)
            prod = cpool.tile([P, E], DT.float32, tag="prod")
            for k in range(KP):
                nc.vector.scalar_tensor_tensor(
                    prod, hk[k], 1.0, emb_own,
                    op0=ALU.mult, op1=ALU.mult, accum_out=d[:, k:k + 1])

            # 16th largest exact dot -> threshold mask
            t8a = cpool.tile([P, 8], DT.float32, tag="t8a")
            t8b = cpool.tile([P, 8], DT.float32, tag="t8b")
            d2 = cpool.tile([P, KPAD], DT.float32, tag="d2")
            nc.vector.max(out=t8a, in_=d)
            nc.vector.match_replace(out=d2, in_to_replace=t8a, in_values=d,
                                    imm_value=NEG)
            nc.vector.max(out=t8b, in_=d2)
            # maskp = (1[d >= thr16] - 1) * 1e9   (0 for kept, -1e9 for dropped)
            maskp = cpool.tile([P, KP], DT.float32, tag="maskp")
            nc.vector.tensor_scalar(maskp, d[:, :KP], t8b[:, 7:8], None,
                                    op0=ALU.is_ge)
            nc.vector.tensor_scalar(maskp, maskp, -1.0, 1.0e9,
                                    op0=ALU.add, op1=ALU.mult)

            # ---------------- attention over the 20-candidate pool -------
            # hT chunks: n = k*128 + t, grouped 4 k's per 512-wide chunk
            for g in range(NGR):
                hTs = [work.tile([P, 512], DT.float32r, tag=f"hTs{kb}",
                                 name=f"hTs{kb}")
                       for kb in range(2)]
                for kk in range(4):
                    k = g * 4 + kk
                    for kb in range(2):
                        pt = ps_tr.tile([P, P], DT.float32, tag="tr")
                        nc.tensor.transpose(out=pt,
                                            in_=hk[k][:, kb * P:(kb + 1) * P],
                                            identity=ident)
                        nc.vector.tensor_copy(hTs[kb][:, kk * P:(kk + 1) * P], pt)
                tanhTs = [work.tile([P, 512], DT.float32r, tag=f"tanhTs{eb}",
                                    name=f"tanhTs{eb}")
                          for eb in range(2)]
                for eb in range(2):
                    pta = ps_chunk.tile([P, 512], DT.float32, tag="chunk", name="pta")
                    for kb in range(2):
                        nc.tensor.matmul(pta, a_r[kb][:, eb * P:(eb + 1) * P],
                                         hTs[kb], start=(kb == 0), stop=(kb == 1))
                    nc.scalar.activation(tanhTs[eb], pta, ACT.Tanh)
                psc = ps_chunk.tile([1, 512], DT.float32, tag="chunk", name="psc")
                for eb in range(2):
                    nc.tensor.matmul(psc, b_r[eb], tanhTs[eb],
                                     start=(eb == 0), stop=(eb == 1))
                scs = work.tile([1, 512], DT.float32, tag="scs")
                nc.vector.tensor_copy(scs, psc)
                nc.sync.dma_start(out=scd[:, g * 512:(g + 1) * 512], in_=scs)

            # scores [t, k] <- scd[k*128 + t]
            sct = cpool.tile([P, KP], DT.float32, tag="sct")
            nc.sync.dma_start(out=sct,
                              in_=scd[:, :].rearrange("o (k t) -> (o t) k", t=TPC))

            # masked softmax over k
            nc.vector.tensor_tensor(sct, sct, maskp, op=ALU.add)
            mx = cpool.tile([P, 1], DT.float32, tag="mx")
            nc.vector.reduce_max(mx, sct, axis=mybir.AxisListType.X)
            negmx = cpool.tile([P, 1], DT.float32, tag="negmx")
            nc.vector.tensor_scalar(negmx, mx, -1.0, None, op0=ALU.mult)
            ex = cpool.tile([P, KP], DT.float32, tag="ex")
            nc.scalar.activation(ex, sct, ACT.Exp, bias=negmx[:, :], scale=1.0)
            sm = cpool.tile([P, 1], DT.float32, tag="sm")
            nc.vector.reduce_sum(sm, ex, axis=mybir.AxisListType.X)
            rc = cpool.tile([P, 1], DT.float32, tag="rc")
            nc.vector.reciprocal(rc, sm)
            att = cpool.tile([P, KP], DT.float32, tag="att_w")
            nc.vector.tensor_scalar(att, ex, rc[:, :], None, op0=ALU.mult)

            # out[t, e] = sum_k att[t,k] * h[t,k,e]
            acc = cpool.tile([P, E], DT.float32, tag="acc")
            nc.vector.memset(acc, 0.0)
            for k in range(KP):
                nc.vector.scalar_tensor_tensor(
                    acc, hk[k], att[:, k:k + 1], acc,
                    op0=ALU.mult, op1=ALU.add)
            nc.sync.dma_start(out=out, in_=acc)

    nc.compile()
    return nc


def get_nc():
    if "v3" not in _BUILD_CACHE:
        _BUILD_CACHE["v3"] = _build()
    return _BUILD_CACHE["v3"]


def kernel(conceptnet_text_vec, table, a, b, topk=16, **_ignored):
    global LAST_RESULTS
    assert int(topk) == TOPK
    tok = np.asarray(conceptnet_text_vec).reshape(NTOK, 1).astype(np.int32)
    table = np.ascontiguousarray(np.asarray(table, dtype=np.float32))
    a = np.ascontiguousarray(np.asarray(a, dtype=np.float32))
    b = np.ascontiguousarray(np.asarray(b, dtype=np.float32)).reshape(E, 1)
    tabT_r = _round12(np.ascontiguousarray(table.T))   # [E, V], f32r-rounded

    nc = get_nc()
    in_maps = []
    for c in range(NCORES):
        in_maps.append({
            "tokidx": tok,
            "tok_own": np.ascontiguousarray(tok[c * TPC:(c + 1) * TPC]),
            "table": table,
            "tabTr": np.ascontiguousarray(tabT_r[:, c * VS:(c + 1) * VS]),
            "amat": a,
            "bvec": b,
        })
    trace = bool(int(os.environ.get("CN_TRACE", "0")))
    res = bass_utils.run_bass_kernel_spmd(nc, in_maps, core_ids=list(range(NCORES)),
                                          trace=trace)
    LAST_RESULTS = res
    outp = np.concatenate([res.results[c]["out"] for c in range(NCORES)], axis=0)
    return outp.reshape(B, L, E)


# revision 16
# speedup vs baseline: 1.1708x; 1.0132x over previous
"""ConceptNet encoder kernel for 8 Trainium2 NeuronCores (Bass/Tile).

Reference computation:
    emb    = table[tok]                      # [1024, 256]
    logits = emb @ table.T                   # [1024, 100000]
    idx    = top16(softmax(logits))          # softmax monotonic -> top16(logits)
    h      = table[idx]                      # [1024, 16, 256]
    e      = tanh(h @ a) @ b                 # [1024, 16]
    out    = softmax(e) @ h                  # [1024, 256]

Distribution: vocab sharded 8 ways. The similarity matmul runs in
float32r (~bf16 speed, ~12-bit mantissa inputs, fp32 accumulate).
Selection packs (quantized value, slot) into one uint32 key per logit
(scalar-engine quantize + one DVE shift-or pass; a few chunks take a
scalar-shift + gpsimd-add path instead to offload the DVE), takes
per-chunk top-8 with a single MAX8 (no FIND_INDEX8, no index plane),
and AllToAll's one key plane per chunk-group (4 pipelined collectives
overlapped with compute). The vocab-shard strips stream through a ring
of SBUF tiles (chunk-group-outer loop), so there is no bulk-load
startup stall. Each core then merges a top-20 candidate pool for its
own 128 tokens, re-scores the pool exactly in fp32 (rescue), and runs
masked-softmax attention over the pool so exactly the true top-16 get
weight.

kernel(**inputs) takes FULL unsharded inputs, returns FULL [4,256,256] output.
Self-contained: hardcodes all shapes; imports only the system concourse repo.
"""
import os
import sys

if "/opt/trn_rl_repo" not in sys.path:
    sys.path.insert(0, "/opt/trn_rl_repo")

import numpy as np

import concourse.bass as bass
import concourse.bacc as bacc
import concourse.mybir as mybir
import concourse.tile as tile
from concourse import bass_utils
from concourse.masks import make_identity

DT = mybir.dt
ALU = mybir.AluOpType
ACT = mybir.ActivationFunctionType

B, L, V, E, TOPK = 4, 256, 100000, 256, 16
NCORES = 8
NTOK = B * L                 # 1024
TPC = NTOK // NCORES         # 128 tokens per core (merge/attention shard)
VS = V // NCORES             # 12500 vocab rows per core
P = 128
NEG = -3.0e38

CW = 1024                    # similarity chunk width (2 PSUM banks)
CHUNKS = []
_off = 0
while _off < VS:
    CHUNKS.append((_off, min(CW, VS - _off)))
    _off += CW
NCHUNK = len(CHUNKS)         # 13 (12x1024 + 212)
PARTS = [[0, 1, 2], [3, 4, 5], [6, 7, 8], [9, 10, 11, 12]]  # j = 3*part + jl
GP_CHUNKS = {1, 4, 7, 11}    # pack pass runs scalar+gpsimd for these chunks
RING = 10                    # streamed tabT strip ring depth (per kb)
NCP = 32                     # candidate slots per (core, part): <=4 chunks x 8
KP = 20                      # rescue pool size per token
KPAD = 24                    # padded pool for max8 rounds
NGR = KP // 4                # attention 512-wide groups
QSCALE = 1536.0              # logit quantizer scale
QBIAS = 3456.0               # makes qi positive (logits in [-2.25, 2])
KEYSHIFT = 11                # slot bits
KEYBASE = 1 << 30            # keeps key bit patterns in normal-float range
MERGEW = 4 * NCORES * NCP    # 1024

_BUILD_CACHE = {}
LAST_RESULTS = None


def _round12(x):
    """Round fp32 to 12 explicit mantissa bits (round half even)."""
    u = np.ascontiguousarray(x, dtype=np.float32).view(np.uint32)
    shift = np.uint32(11)
    mask = np.uint32((1 << 11) - 1)
    half = np.uint32(1 << 10)
    frac = u & mask
    u2 = u & ~mask
    rnd = (frac > half) | ((frac == half) & (((u2 >> shift) & np.uint32(1)) == 1))
    u2 = u2 + (rnd.astype(np.uint32) << shift)
    return u2.view(np.float32)


def _build():
    nc = bacc.Bacc("TRN2", target_bir_lowering=False, debug=False,
                   enable_asserts=True, num_devices=NCORES)

    tokidx = nc.dram_tensor("tokidx", [NTOK, 1], DT.int32, kind="ExternalInput").ap()
    tok_own = nc.dram_tensor("tok_own", [TPC, 1], DT.int32, kind="ExternalInput").ap()
    table = nc.dram_tensor("table", [V, E], DT.float32, kind="ExternalInput").ap()
    tabTr = nc.dram_tensor("tabTr", [E, VS], DT.float32r, kind="ExternalInput").ap()
    amat = nc.dram_tensor("amat", [E, E], DT.float32, kind="ExternalInput").ap()
    bvec = nc.dram_tensor("bvec", [E, 1], DT.float32, kind="ExternalInput").ap()
    out = nc.dram_tensor("out", [TPC, E], DT.float32, kind="ExternalOutput").ap()

    with tile.TileContext(nc) as tc:
        with tc.tile_pool(name="const", bufs=1) as cpool, \
             tc.tile_pool(name="big", bufs=1) as big, \
             tc.tile_pool(name="work", bufs=2) as work, \
             tc.tile_pool(name="ps_chunk", bufs=3, space="PSUM") as ps_chunk, \
             tc.tile_pool(name="ps_tr", bufs=2, space="PSUM") as ps_tr, \
             tc.tile_pool(name="dram", bufs=1, space="DRAM") as dram:

            # ---------------- constants ----------------
            ident = cpool.tile([P, P], DT.float32, tag="ident")
            make_identity(nc, ident)

            iotaK = cpool.tile([P, CW], DT.uint32, tag="iotaK")
            nc.gpsimd.iota(iotaK, pattern=[[1, CW]], base=KEYBASE,
                           channel_multiplier=0)

            def const_col(name, val):
                t = cpool.tile([P, 1], DT.uint32, tag=name, name=name)
                nc.gpsimd.iota(t, pattern=[[0, 1]], base=val, channel_multiplier=0)
                return t

            c_shift = const_col("c_shift", KEYSHIFT)
            c_slotmask = const_col("c_slotmask", (1 << KEYSHIFT) - 1)
            c_8 = const_col("c_8", 8)
            c_5 = const_col("c_5", 5)
            c_7 = const_col("c_7", 7)
            c_3 = const_col("c_3", 3)

            # ---------------- emb gather + f32r transpose ----------------
            embT = [[big.tile([P, P], DT.float32r, tag=f"embT{kb}_{m}",
                              name=f"embT{kb}_{m}")
                     for m in range(NCORES)] for kb in range(2)]
            for m in range(NCORES):
                ti = work.tile([P, 1], DT.int32, tag="ti")
                nc.sync.dma_start(out=ti, in_=tokidx[m * P:(m + 1) * P, :])
                em = work.tile([P, E], DT.float32, tag="em")
                nc.gpsimd.indirect_dma_start(
                    out=em, out_offset=None, in_=table,
                    in_offset=bass.IndirectOffsetOnAxis(ap=ti[:, :], axis=0))
                for kb in range(2):
                    pt = ps_tr.tile([P, P], DT.float32, tag="tr")
                    nc.tensor.transpose(out=pt, in_=em[:, kb * P:(kb + 1) * P],
                                        identity=ident)
                    nc.vector.tensor_copy(embT[kb][m], pt)

            # own-token embeddings (fp32, for exact rescue dots)
            ti_own = cpool.tile([P, 1], DT.int32, tag="ti_own")
            nc.sync.dma_start(out=ti_own, in_=tok_own)
            emb_own = cpool.tile([P, E], DT.float32, tag="emb_own")
            nc.gpsimd.indirect_dma_start(
                out=emb_own, out_offset=None, in_=table,
                in_offset=bass.IndirectOffsetOnAxis(ap=ti_own[:, :], axis=0))

            # ---------------- small weights ----------------
            a_sb = []
            for kb in range(2):
                t = cpool.tile([P, E], DT.float32, tag=f"a{kb}", name=f"a{kb}")
                nc.sync.dma_start(out=t, in_=amat[kb * P:(kb + 1) * P, :])
                a_sb.append(t)
            a_r = []
            for kb in range(2):
                t = cpool.tile([P, E], DT.float32r, tag=f"ar{kb}", name=f"ar{kb}")
                nc.vector.tensor_copy(t, a_sb[kb])
                a_r.append(t)
            b_sb = []
            for kb in range(2):
                t = cpool.tile([P, 1], DT.float32, tag=f"b{kb}", name=f"b{kb}")
                nc.sync.dma_start(out=t, in_=bvec[kb * P:(kb + 1) * P, :])
                b_sb.append(t)
            b_r = []
            for kb in range(2):
                t = cpool.tile([P, 1], DT.float32r, tag=f"br{kb}", name=f"br{kb}")
                nc.vector.tensor_copy(t, b_sb[kb])
                b_r.append(t)

            # ---------------- streamed tabT strips ----------------
            strip = {}

            def issue_strips(part):
                for j in PARTS[part]:
                    off, w = CHUNKS[j]
                    for kb in range(2):
                        t = big.tile([P, w], DT.float32r, tag=f"tt{kb}_{j % RING}",
                                     name=f"tt{kb}_{j % RING}")
                        nc.sync.dma_start(out=t, in_=tabTr[kb * P:(kb + 1) * P,
                                                           off:off + w])
                        strip[(kb, j)] = t

            issue_strips(0)
            issue_strips(1)

            vals = cpool.tile([P, MERGEW], DT.float32, tag="vals")

            def load_vals(part):
                # vals[p, part*256 + c*32 + s] = agg[part][(c, p, s)]
                agg_v = agg[part][:, :].rearrange("(c p s) o -> c p (s o)",
                                                  c=NCORES, p=TPC)
                for c in range(NCORES):
                    o = part * NCORES * NCP + c * NCP
                    nc.gpsimd.dma_start(out=vals[:, o:o + NCP], in_=agg_v[c])

            # ---------------- a2a bounce buffers ----------------
            bounce = [dram.tile([NCORES, TPC, NCP], DT.float32, tag=f"bounce{p}",
                                name=f"bounce{p}")
                      for p in range(4)]
            agg = [dram.tile([NCORES * TPC * NCP, 1], DT.float32, tag=f"agg{p}",
                             name=f"agg{p}")
                   for p in range(4)]
            scd = dram.tile([1, TPC * KP], DT.float32, tag="scd")

            # ---------------- similarity + packed per-chunk top-8 --------
            for part in range(4):
                if part + 2 < 4:
                    issue_strips(part + 2)
                pjs = PARTS[part]
                for m in range(NCORES):
                    if m == 1 and part >= 1:
                        nc.gpsimd.collective_compute(
                            "AllToAll", ALU.bypass,
                            replica_groups=[list(range(NCORES))],
                            ins=[bounce[part - 1][:, :, :].opt()],
                            outs=[agg[part - 1][:, :].opt()],
                        )
                    if m == 5 and part >= 2:
                        load_vals(part - 2)
                    cv = work.tile([P, NCP], DT.float32, tag="cv")
                    if len(pjs) < 4:
                        nc.vector.memset(cv[:, len(pjs) * 8:], 0.0)
                    for jl, j in enumerate(pjs):
                        off, w = CHUNKS[j]
                        ps = ps_chunk.tile([P, CW], DT.float32, tag="chunk")
                        for kb in range(2):
                            for h in range((w + 511) // 512):
                                hw = min(512, w - h * 512)
                                nc.tensor.matmul(
                                    ps[:, h * 512:h * 512 + hw],
                                    embT[kb][m],
                                    strip[(kb, j)][:, h * 512:h * 512 + hw],
                                    start=(kb == 0), stop=(kb == 1))
                        keys = work.tile([P, CW], DT.uint32, tag="keys", bufs=4)
                        if j in GP_CHUNKS:
                            # scalar: quantize + exact *2048 shift; gpsimd: +iota
                            q1 = work.tile([P, CW], DT.int32, tag="q1", bufs=3)
                            nc.scalar.activation(q1[:, :w], ps[:, :w],
                                                 ACT.Copy, scale=QSCALE,
                                                 bias=QBIAS)
                            nc.scalar.activation(keys[:, :w].bitcast(DT.int32),
                                                 q1[:, :w], ACT.Copy,
                                                 scale=float(1 << KEYSHIFT))
                            nc.gpsimd.tensor_tensor(keys[:, :w], keys[:, :w],
                                                    iotaK[:, :w], op=ALU.add)
                        else:
                            # quantize logits -> int (scalar engine reads PSUM)
                            nc.scalar.activation(keys[:, :w].bitcast(DT.int32),
                                                 ps[:, :w], ACT.Copy,
                                                 scale=QSCALE, bias=QBIAS)
                            # key = (qi << 11) | slot | 2^30  (one DVE pass)
                            nc.vector.scalar_tensor_tensor(
                                keys[:, :w], keys[:, :w], c_shift[:, :],
                                iotaK[:, :w],
                                op0=ALU.logical_shift_left, op1=ALU.bitwise_or)
                        nc.vector.max(out=cv[:, jl * 8:(jl + 1) * 8],
                                      in_=keys[:, :w].bitcast(DT.float32))
                    nc.sync.dma_start(out=bounce[part][m, :, :], in_=cv)

            nc.gpsimd.collective_compute(
                "AllToAll", ALU.bypass,
                replica_groups=[list(range(NCORES))],
                ins=[bounce[3][:, :, :].opt()],
                outs=[agg[3][:, :].opt()],
            )

            load_vals(2)
            load_vals(3)

            # ---------------- merge: top-20 keys + positions -------------
            wk = cpool.tile([P, KPAD], DT.float32, tag="wk")
            wp = cpool.tile([P, KPAD], DT.uint32, tag="wp")
            vals2 = cpool.tile([P, MERGEW], DT.float32, tag="vals2")
            vals3 = cpool.tile([P, MERGEW], DT.float32, tag="vals3")

            # ---------------- decode global vocab indices ----------------
            # pos = part*256 + c*32 + jl*8 + r ; key = (qi<<11)|slot|2^30
            slot = cpool.tile([P, KPAD], DT.uint32, tag="slot", name="slot")
            prt = cpool.tile([P, KPAD], DT.uint32, tag="prt", name="prt")
            csrc = cpool.tile([P, KPAD], DT.uint32, tag="csrc", name="csrc")
            jl = cpool.tile([P, KPAD], DT.uint32, tag="jl", name="jl")
            gidx = cpool.tile([P, KPAD], DT.uint32, tag="gidx", name="gidx")
            t2 = cpool.tile([P, KPAD], DT.uint32, tag="t2", name="t2")
            hk = [cpool.tile([P, E], DT.float32, tag=f"h{k}", name=f"h{k}")
                  for k in range(KP)]

            def decode_and_gather(g0, g1):
                """Decode candidate slots [g0,g1) and launch their h gathers."""
                gs = slice(g0, g1)
                nc.vector.tensor_scalar(slot[:, gs], wk[:, gs].bitcast(DT.uint32),
                                        c_slotmask[:, :], None,
                                        op0=ALU.bitwise_and)
                nc.vector.tensor_scalar(prt[:, gs], wp[:, gs], c_8[:, :], None,
                                        op0=ALU.logical_shift_right)
                nc.vector.tensor_scalar(csrc[:, gs], wp[:, gs], c_5[:, :], None,
                                        op0=ALU.logical_shift_right)
                nc.vector.tensor_scalar(csrc[:, gs], csrc[:, gs], c_7[:, :], None,
                                        op0=ALU.bitwise_and)
                nc.vector.tensor_scalar(jl[:, gs], wp[:, gs], c_3[:, :], None,
                                        op0=ALU.logical_shift_right)
                nc.vector.tensor_scalar(jl[:, gs], jl[:, gs], c_3[:, :], None,
                                        op0=ALU.bitwise_and)
                # gidx = csrc*12500 + (3*part + jl)*1024 + slot (< 2^24: fp-exact)
                nc.vector.tensor_scalar(gidx[:, gs], csrc[:, gs], float(VS),
                                        None, op0=ALU.mult)
                nc.vector.tensor_scalar(t2[:, gs], prt[:, gs], 3.0 * CW, None,
                                        op0=ALU.mult)
                nc.vector.tensor_tensor(gidx[:, gs], gidx[:, gs], t2[:, gs],
                                        op=ALU.add)
                nc.vector.tensor_scalar(t2[:, gs], jl[:, gs], float(CW), None,
                                        op0=ALU.mult)
                nc.vector.tensor_tensor(gidx[:, gs], gidx[:, gs], t2[:, gs],
                                        op=ALU.add)
                nc.vector.tensor_tensor(gidx[:, gs], gidx[:, gs], slot[:, gs],
                                        op=ALU.add)
                for k in range(g0, min(g1, KP)):
                    nc.gpsimd.indirect_dma_start(
                        out=hk[k], out_offset=None, in_=table,
                        in_offset=bass.IndirectOffsetOnAxis(
                            ap=gidx[:, :].bitcast(DT.int32)[:, k:k + 1], axis=0))

            nc.vector.max(out=wk[:, 0:8], in_=vals)
            nc.vector.max_index(out=wp[:, 0:8], in_max=wk[:, 0:8], in_values=vals)
            nc.vector.match_replace(out=vals2, in_to_replace=wk[:, 0:8],
                                    in_values=vals, imm_value=0.0)
            decode_and_gather(0, 8)
            nc.vector.max(out=wk[:, 8:16], in_=vals2)
            nc.vector.max_index(out=wp[:, 8:16], in_max=wk[:, 8:16], in_values=vals2)
            nc.vector.match_replace(out=vals3, in_to_replace=wk[:, 8:16],
                                    in_values=vals2, imm_value=0.0)
            decode_and_gather(8, 16)
            nc.vector.max(out=wk[:, 16:24], in_=vals3)
            nc.vector.max_index(out=wp[:, 16:24], in_max=wk[:, 16:24], in_values=vals3)
            decode_and_gather(16, KP)

            d = cpool.tile([P, KPAD], DT.float32, tag="d")
            nc.vector.memset(d[:, KP:], NEG)
            prod = cpool.tile([P, E], DT.float32, tag="prod")
            for k in range(KP):
                nc.vector.scalar_tensor_tensor(
                    prod, hk[k], 1.0, emb_own,
                    op0=ALU.mult, op1=ALU.mult, accum_out=d[:, k:k + 1])

            # 16th largest exact dot -> threshold mask
            t8a = cpool.tile([P, 8], DT.float32, tag="t8a")
            t8b = cpool.tile([P, 8], DT.float32, tag="t8b")
            d2 = cpool.tile([P, KPAD], DT.float32, tag="d2")
            nc.vector.max(out=t8a, in_=d)
            nc.vector.match_replace(out=d2, in_to_replace=t8a, in_values=d,
                                    imm_value=NEG)
            nc.vector.max(out=t8b, in_=d2)
            # maskp = (1[d >= thr16] - 1) * 1e9   (0 for kept, -1e9 for dropped)
            maskp = cpool.tile([P, KP], DT.float32, tag="maskp")
            nc.vector.tensor_scalar(maskp, d[:, :KP], t8b[:, 7:8], None,
                                    op0=ALU.is_ge)
            nc.vector.tensor_scalar(maskp, maskp, -1.0, 1.0e9,
                                    op0=ALU.add, op1=ALU.mult)

            # ---------------- attention over the 20-candidate pool -------
            # hT chunks: n = k*128 + t, grouped 4 k's per 512-wide chunk
            for g in range(NGR):
                hTs = [work.tile([P, 512], DT.float32r, tag=f"hTs{kb}",
                                 name=f"hTs{kb}")
                       for kb in range(2)]
                for kk in range(4):
                    k = g * 4 + kk
                    for kb in range(2):
                        pt = ps_tr.tile([P, P], DT.float32, tag="tr")
                        nc.tensor.transpose(out=pt,
                                            in_=hk[k][:, kb * P:(kb + 1) * P],
                                            identity=ident)
                        nc.vector.tensor_copy(hTs[kb][:, kk * P:(kk + 1) * P], pt)
                tanhTs = [work.tile([P, 512], DT.float32r, tag=f"tanhTs{eb}",
                                    name=f"tanhTs{eb}")
                          for eb in range(2)]
                for eb in range(2):
                    pta = ps_chunk.tile([P, 512], DT.float32, tag="chunk", name="pta")
                    for kb in range(2):
                        nc.tensor.matmul(pta, a_r[kb][:, eb * P:(eb + 1) * P],
                                         hTs[kb], start=(kb == 0), stop=(kb == 1))
                    nc.scalar.activation(tanhTs[eb], pta, ACT.Tanh)
                psc = ps_chunk.tile([1, 512], DT.float32, tag="chunk", name="psc")
                for eb in range(2):
                    nc.tensor.matmul(psc, b_r[eb], tanhTs[eb],
                                     start=(eb == 0), stop=(eb == 1))
                scs = work.tile([1, 512], DT.float32, tag="scs")
                nc.vector.tensor_copy(scs, psc)
                nc.sync.dma_start(out=scd[:, g * 512:(g + 1) * 512], in_=scs)

            # scores [t, k] <- scd[k*128 + t]
            sct = cpool.tile([P, KP], DT.float32, tag="sct")
            for g in range(NGR):
                nc.sync.dma_start(
                    out=sct[:, g * 4:(g + 1) * 4],
                    in_=scd[:, g * 512:(g + 1) * 512].rearrange(
                        "o (k t) -> (o t) k", t=TPC))

            # masked softmax over k
            nc.vector.tensor_tensor(sct, sct, maskp, op=ALU.add)
            mx = cpool.tile([P, 1], DT.float32, tag="mx")
            nc.vector.reduce_max(mx, sct, axis=mybir.AxisListType.X)
            negmx = cpool.tile([P, 1], DT.float32, tag="negmx")
            nc.vector.tensor_scalar(negmx, mx, -1.0, None, op0=ALU.mult)
            ex = cpool.tile([P, KP], DT.float32, tag="ex")
            nc.scalar.activation(ex, sct, ACT.Exp, bias=negmx[:, :], scale=1.0)
            sm = cpool.tile([P, 1], DT.float32, tag="sm")
            nc.vector.reduce_sum(sm, ex, axis=mybir.AxisListType.X)
            rc = cpool.tile([P, 1], DT.float32, tag="rc")
            nc.vector.reciprocal(rc, sm)
            att = cpool.tile([P, KP], DT.float32, tag="att_w")
            nc.vector.tensor_scalar(att, ex, rc[:, :], None, op0=ALU.mult)

            # out[t, e] = sum_k att[t,k] * h[t,k,e]
            acc = cpool.tile([P, E], DT.float32, tag="acc")
            accB = cpool.tile([P, E], DT.float32, tag="accB")
            term = cpool.tile([P, E], DT.float32, tag="term", bufs=3)
            nc.vector.memset(acc, 0.0)
            nc.vector.memset(accB, 0.0)
            for k in range(KP):
                if k % 2 == 0:
                    nc.vector.scalar_tensor_tensor(
                        acc, hk[k], att[:, k:k + 1], acc,
                        op0=ALU.mult, op1=ALU.add)
                else:
                    tk = cpool.tile([P, E], DT.float32, tag="term", bufs=3,
                                    name="tk")
                    nc.scalar.activation(tk, hk[k], ACT.Copy,
                                         scale=att[:, k:k + 1])
                    nc.vector.tensor_tensor(accB, accB, tk, op=ALU.add)
            nc.vector.tensor_tensor(acc, acc, accB, op=ALU.add)
            nc.sync.dma_start(out=out, in_=acc)

    nc.compile()
    return nc


def get_nc():
    if "v3" not in _BUILD_CACHE:
        _BUILD_CACHE["v3"] = _build()
    return _BUILD_CACHE["v3"]


def kernel(conceptnet_text_vec, table, a, b, topk=16, **_ignored):
    global LAST_RESULTS
    assert int(topk) == TOPK
    tok = np.asarray(conceptnet_text_vec).reshape(NTOK, 1).astype(np.int32)
    table = np.ascontiguousarray(np.asarray(table, dtype=np.float32))
    a = np.ascontiguousarray(np.asarray(a, dtype=np.float32))
    b = np.ascontiguousarray(np.asarray(b, dtype=np.float32)).reshape(E, 1)
    tabT_r = _round12(np.ascontiguousarray(table.T))   # [E, V], f32r-rounded

    nc = get_nc()
    in_maps = []
    for c in range(NCORES):
        in_maps.append({
            "tokidx": tok,
            "tok_own": np.ascontiguousarray(tok[c * TPC:(c + 1) * TPC]),
            "table": table,
            "tabTr": np.ascontiguousarray(tabT_r[:, c * VS:(c + 1) * VS]),
            "amat": a,
            "bvec": b,
        })
    trace = bool(int(os.environ.get("CN_TRACE", "0")))
    res = bass_utils.run_bass_kernel_spmd(nc, in_maps, core_ids=list(range(NCORES)),
                                          trace=trace)
    LAST_RESULTS = res
    outp = np.concatenate([res.results[c]["out"] for c in range(NCORES)], axis=0)
    return outp.reshape(B, L, E)


# revision 17
# speedup vs baseline: 1.2326x; 1.0528x over previous
"""ConceptNet encoder kernel for 8 Trainium2 NeuronCores (Bass/Tile).

Reference computation:
    emb    = table[tok]                      # [1024, 256]
    logits = emb @ table.T                   # [1024, 100000]
    idx    = top16(softmax(logits))          # softmax monotonic -> top16(logits)
    h      = table[idx]                      # [1024, 16, 256]
    e      = tanh(h @ a) @ b                 # [1024, 16]
    out    = softmax(e) @ h                  # [1024, 256]

Distribution: vocab sharded 8 ways. The similarity matmul runs in
float32r (~bf16 speed, ~12-bit mantissa inputs, fp32 accumulate).
Selection packs (quantized value, slot) into one uint32 key per logit
(scalar-engine quantize + one DVE shift-or pass; a few chunks take a
scalar-shift + gpsimd-add path instead to offload the DVE), takes
per-chunk top-8 with a single MAX8 (no FIND_INDEX8, no index plane),
and AllToAll's one key plane per chunk-group (4 pipelined collectives
overlapped with compute). The vocab-shard strips stream through a ring
of SBUF tiles (chunk-group-outer loop), so there is no bulk-load
startup stall. Each core then merges a top-20 candidate pool for its
own 128 tokens, re-scores the pool exactly in fp32 (rescue), and runs
masked-softmax attention over the pool so exactly the true top-16 get
weight.

kernel(**inputs) takes FULL unsharded inputs, returns FULL [4,256,256] output.
Self-contained: hardcodes all shapes; imports only the system concourse repo.
"""
import os
import sys

if "/opt/trn_rl_repo" not in sys.path:
    sys.path.insert(0, "/opt/trn_rl_repo")

import numpy as np

import concourse.bass as bass
import concourse.bacc as bacc
import concourse.mybir as mybir
import concourse.tile as tile
from concourse import bass_utils
from concourse.masks import make_identity

DT = mybir.dt
ALU = mybir.AluOpType
ACT = mybir.ActivationFunctionType

B, L, V, E, TOPK = 4, 256, 100000, 256, 16
NCORES = 8
NTOK = B * L                 # 1024
TPC = NTOK // NCORES         # 128 tokens per core (merge/attention shard)
VS = V // NCORES             # 12500 vocab rows per core
P = 128
NEG = -3.0e38

CW = 1024                    # similarity chunk width (2 PSUM banks)
CHUNKS = []
_off = 0
while _off < VS:
    CHUNKS.append((_off, min(CW, VS - _off)))
    _off += CW
NCHUNK = len(CHUNKS)         # 13 (12x1024 + 212)
PARTS = [[0, 1, 2], [3, 4, 5], [6, 7, 8], [9, 10, 11, 12]]  # j = 3*part + jl
GP_CHUNKS = {0, 1, 2}        # gpsimd-assisted pack: part 0 only (collectives
                             # block the gpsimd queue once they are issued)
RING = 10                    # streamed tabT strip ring depth (per kb)
NCP = 32                     # candidate slots per (core, part): <=4 chunks x 8
KP = 20                      # rescue pool size per token
KPAD = 24                    # padded pool for max8 rounds
NGR = KP // 4                # attention 512-wide groups
QSCALE = 1536.0              # logit quantizer scale
QBIAS = 3456.0               # makes qi positive (logits in [-2.25, 2])
KEYSHIFT = 11                # slot bits
KEYBASE = 1 << 30            # keeps key bit patterns in normal-float range
MERGEW = 4 * NCORES * NCP    # 1024

_BUILD_CACHE = {}
LAST_RESULTS = None


def _round12(x):
    """Round fp32 to 12 explicit mantissa bits (round half even)."""
    u = np.ascontiguousarray(x, dtype=np.float32).view(np.uint32)
    shift = np.uint32(11)
    mask = np.uint32((1 << 11) - 1)
    half = np.uint32(1 << 10)
    frac = u & mask
    u2 = u & ~mask
    rnd = (frac > half) | ((frac == half) & (((u2 >> shift) & np.uint32(1)) == 1))
    u2 = u2 + (rnd.astype(np.uint32) << shift)
    return u2.view(np.float32)


def _build():
    nc = bacc.Bacc("TRN2", target_bir_lowering=False, debug=False,
                   enable_asserts=True, num_devices=NCORES)

    tokidx = nc.dram_tensor("tokidx", [NTOK, 1], DT.int32, kind="ExternalInput").ap()
    tok_own = nc.dram_tensor("tok_own", [TPC, 1], DT.int32, kind="ExternalInput").ap()
    table = nc.dram_tensor("table", [V, E], DT.float32, kind="ExternalInput").ap()
    tabTr = nc.dram_tensor("tabTr", [E, VS], DT.float32r, kind="ExternalInput").ap()
    amat = nc.dram_tensor("amat", [E, E], DT.float32, kind="ExternalInput").ap()
    bvec = nc.dram_tensor("bvec", [E, 1], DT.float32, kind="ExternalInput").ap()
    out = nc.dram_tensor("out", [TPC, E], DT.float32, kind="ExternalOutput").ap()

    with tile.TileContext(nc) as tc:
        with tc.tile_pool(name="const", bufs=1) as cpool, \
             tc.tile_pool(name="big", bufs=1) as big, \
             tc.tile_pool(name="work", bufs=2) as work, \
             tc.tile_pool(name="ps_chunk", bufs=3, space="PSUM") as ps_chunk, \
             tc.tile_pool(name="ps_tr", bufs=2, space="PSUM") as ps_tr, \
             tc.tile_pool(name="dram", bufs=1, space="DRAM") as dram:

            # ---------------- constants ----------------
            ident = cpool.tile([P, P], DT.float32, tag="ident")
            make_identity(nc, ident)

            iotaK = cpool.tile([P, CW], DT.uint32, tag="iotaK")
            nc.gpsimd.iota(iotaK, pattern=[[1, CW]], base=KEYBASE,
                           channel_multiplier=0)

            def const_col(name, val):
                t = cpool.tile([P, 1], DT.uint32, tag=name, name=name)
                nc.gpsimd.iota(t, pattern=[[0, 1]], base=val, channel_multiplier=0)
                return t

            c_shift = const_col("c_shift", KEYSHIFT)
            c_slotmask = const_col("c_slotmask", (1 << KEYSHIFT) - 1)
            c_8 = const_col("c_8", 8)
            c_5 = const_col("c_5", 5)
            c_7 = const_col("c_7", 7)
            c_3 = const_col("c_3", 3)

            # ---------------- emb gather + f32r transpose ----------------
            embT = [[big.tile([P, P], DT.float32r, tag=f"embT{kb}_{m}",
                              name=f"embT{kb}_{m}")
                     for m in range(NCORES)] for kb in range(2)]
            for m in range(NCORES):
                ti = work.tile([P, 1], DT.int32, tag="ti")
                nc.sync.dma_start(out=ti, in_=tokidx[m * P:(m + 1) * P, :])
                em = work.tile([P, E], DT.float32, tag="em")
                nc.gpsimd.indirect_dma_start(
                    out=em, out_offset=None, in_=table,
                    in_offset=bass.IndirectOffsetOnAxis(ap=ti[:, :], axis=0))
                for kb in range(2):
                    pt = ps_tr.tile([P, P], DT.float32, tag="tr")
                    nc.tensor.transpose(out=pt, in_=em[:, kb * P:(kb + 1) * P],
                                        identity=ident)
                    nc.vector.tensor_copy(embT[kb][m], pt)

            # own-token embeddings (fp32, for exact rescue dots)
            ti_own = cpool.tile([P, 1], DT.int32, tag="ti_own")
            nc.sync.dma_start(out=ti_own, in_=tok_own)
            emb_own = cpool.tile([P, E], DT.float32, tag="emb_own")
            nc.gpsimd.indirect_dma_start(
                out=emb_own, out_offset=None, in_=table,
                in_offset=bass.IndirectOffsetOnAxis(ap=ti_own[:, :], axis=0))

            # ---------------- small weights ----------------
            a_sb = []
            for kb in range(2):
                t = cpool.tile([P, E], DT.float32, tag=f"a{kb}", name=f"a{kb}")
                nc.sync.dma_start(out=t, in_=amat[kb * P:(kb + 1) * P, :])
                a_sb.append(t)
            a_r = []
            for kb in range(2):
                t = cpool.tile([P, E], DT.float32r, tag=f"ar{kb}", name=f"ar{kb}")
                nc.vector.tensor_copy(t, a_sb[kb])
                a_r.append(t)
            b_sb = []
            for kb in range(2):
                t = cpool.tile([P, 1], DT.float32, tag=f"b{kb}", name=f"b{kb}")
                nc.sync.dma_start(out=t, in_=bvec[kb * P:(kb + 1) * P, :])
                b_sb.append(t)
            b_r = []
            for kb in range(2):
                t = cpool.tile([P, 1], DT.float32r, tag=f"br{kb}", name=f"br{kb}")
                nc.vector.tensor_copy(t, b_sb[kb])
                b_r.append(t)

            # ---------------- streamed tabT strips ----------------
            strip = {}

            def issue_strips(part):
                for j in PARTS[part]:
                    off, w = CHUNKS[j]
                    for kb in range(2):
                        t = big.tile([P, w], DT.float32r, tag=f"tt{kb}_{j % RING}",
                                     name=f"tt{kb}_{j % RING}")
                        nc.sync.dma_start(out=t, in_=tabTr[kb * P:(kb + 1) * P,
                                                           off:off + w])
                        strip[(kb, j)] = t

            issue_strips(0)
            issue_strips(1)

            vals = cpool.tile([P, MERGEW], DT.float32, tag="vals")

            def load_vals(part):
                # vals[p, part*256 + c*32 + s] = agg[part][(c, p, s)]
                agg_v = agg[part][:, :].rearrange("(c p s) o -> c p (s o)",
                                                  c=NCORES, p=TPC)
                for c in range(NCORES):
                    o = part * NCORES * NCP + c * NCP
                    nc.gpsimd.dma_start(out=vals[:, o:o + NCP], in_=agg_v[c])

            # ---------------- a2a bounce buffers ----------------
            bounce = [dram.tile([NCORES, TPC, NCP], DT.float32, tag=f"bounce{p}",
                                name=f"bounce{p}")
                      for p in range(4)]
            agg = [dram.tile([NCORES * TPC * NCP, 1], DT.float32, tag=f"agg{p}",
                             name=f"agg{p}")
                   for p in range(4)]
            scd = dram.tile([1, TPC * KP], DT.float32, tag="scd")

            # ---------------- similarity + packed per-chunk top-8 --------
            for part in range(4):
                if part + 2 < 4:
                    issue_strips(part + 2)
                pjs = PARTS[part]
                for m in range(NCORES):
                    if m == 1 and part >= 1:
                        nc.gpsimd.collective_compute(
                            "AllToAll", ALU.bypass,
                            replica_groups=[list(range(NCORES))],
                            ins=[bounce[part - 1][:, :, :].opt()],
                            outs=[agg[part - 1][:, :].opt()],
                        )
                    if m == 5 and part >= 2:
                        load_vals(part - 2)
                    cv = work.tile([P, NCP], DT.float32, tag="cv")
                    if len(pjs) < 4:
                        nc.vector.memset(cv[:, len(pjs) * 8:], 0.0)
                    for jl, j in enumerate(pjs):
                        off, w = CHUNKS[j]
                        ps = ps_chunk.tile([P, CW], DT.float32, tag="chunk")
                        for kb in range(2):
                            for h in range((w + 511) // 512):
                                hw = min(512, w - h * 512)
                                nc.tensor.matmul(
                                    ps[:, h * 512:h * 512 + hw],
                                    embT[kb][m],
                                    strip[(kb, j)][:, h * 512:h * 512 + hw],
                                    start=(kb == 0), stop=(kb == 1))
                        keys = work.tile([P, CW], DT.uint32, tag="keys", bufs=4)
                        if j in GP_CHUNKS:
                            # scalar: quantize + exact *2048 shift; gpsimd: +iota
                            q1 = work.tile([P, CW], DT.int32, tag="q1", bufs=3)
                            nc.scalar.activation(q1[:, :w], ps[:, :w],
                                                 ACT.Copy, scale=QSCALE,
                                                 bias=QBIAS)
                            nc.scalar.activation(keys[:, :w].bitcast(DT.int32),
                                                 q1[:, :w], ACT.Copy,
                                                 scale=float(1 << KEYSHIFT))
                            nc.gpsimd.tensor_tensor(keys[:, :w], keys[:, :w],
                                                    iotaK[:, :w], op=ALU.add)
                        else:
                            # quantize logits -> int (scalar engine reads PSUM)
                            nc.scalar.activation(keys[:, :w].bitcast(DT.int32),
                                                 ps[:, :w], ACT.Copy,
                                                 scale=QSCALE, bias=QBIAS)
                            # key = (qi << 11) | slot | 2^30  (one DVE pass)
                            nc.vector.scalar_tensor_tensor(
                                keys[:, :w], keys[:, :w], c_shift[:, :],
                                iotaK[:, :w],
                                op0=ALU.logical_shift_left, op1=ALU.bitwise_or)
                        nc.vector.max(out=cv[:, jl * 8:(jl + 1) * 8],
                                      in_=keys[:, :w].bitcast(DT.float32))
                    nc.sync.dma_start(out=bounce[part][m, :, :], in_=cv)

            nc.gpsimd.collective_compute(
                "AllToAll", ALU.bypass,
                replica_groups=[list(range(NCORES))],
                ins=[bounce[3][:, :, :].opt()],
                outs=[agg[3][:, :].opt()],
            )

            load_vals(2)
            load_vals(3)

            # ---------------- merge: top-20 keys + positions -------------
            wk = cpool.tile([P, KPAD], DT.float32, tag="wk")
            wp = cpool.tile([P, KPAD], DT.uint32, tag="wp")
            vals2 = cpool.tile([P, MERGEW], DT.float32, tag="vals2")
            vals3 = cpool.tile([P, MERGEW], DT.float32, tag="vals3")

            # ---------------- decode global vocab indices ----------------
            # pos = part*256 + c*32 + jl*8 + r ; key = (qi<<11)|slot|2^30
            slot = cpool.tile([P, KPAD], DT.uint32, tag="slot", name="slot")
            prt = cpool.tile([P, KPAD], DT.uint32, tag="prt", name="prt")
            csrc = cpool.tile([P, KPAD], DT.uint32, tag="csrc", name="csrc")
            jl = cpool.tile([P, KPAD], DT.uint32, tag="jl", name="jl")
            gidx = cpool.tile([P, KPAD], DT.uint32, tag="gidx", name="gidx")
            t2 = cpool.tile([P, KPAD], DT.uint32, tag="t2", name="t2")
            hk = [cpool.tile([P, E], DT.float32, tag=f"h{k}", name=f"h{k}")
                  for k in range(KP)]

            def decode_and_gather(g0, g1):
                """Decode candidate slots [g0,g1) and launch their h gathers."""
                gs = slice(g0, g1)
                nc.vector.tensor_scalar(slot[:, gs], wk[:, gs].bitcast(DT.uint32),
                                        c_slotmask[:, :], None,
                                        op0=ALU.bitwise_and)
                nc.vector.tensor_scalar(prt[:, gs], wp[:, gs], c_8[:, :], None,
                                        op0=ALU.logical_shift_right)
                nc.vector.tensor_scalar(csrc[:, gs], wp[:, gs], c_5[:, :], None,
                                        op0=ALU.logical_shift_right)
                nc.vector.tensor_scalar(csrc[:, gs], csrc[:, gs], c_7[:, :], None,
                                        op0=ALU.bitwise_and)
                nc.vector.tensor_scalar(jl[:, gs], wp[:, gs], c_3[:, :], None,
                                        op0=ALU.logical_shift_right)
                nc.vector.tensor_scalar(jl[:, gs], jl[:, gs], c_3[:, :], None,
                                        op0=ALU.bitwise_and)
                # gidx = csrc*12500 + (3*part + jl)*1024 + slot (< 2^24: fp-exact)
                nc.vector.tensor_scalar(gidx[:, gs], csrc[:, gs], float(VS),
                                        None, op0=ALU.mult)
                nc.vector.tensor_scalar(t2[:, gs], prt[:, gs], 3.0 * CW, None,
                                        op0=ALU.mult)
                nc.vector.tensor_tensor(gidx[:, gs], gidx[:, gs], t2[:, gs],
                                        op=ALU.add)
                nc.vector.tensor_scalar(t2[:, gs], jl[:, gs], float(CW), None,
                                        op0=ALU.mult)
                nc.vector.tensor_tensor(gidx[:, gs], gidx[:, gs], t2[:, gs],
                                        op=ALU.add)
                nc.vector.tensor_tensor(gidx[:, gs], gidx[:, gs], slot[:, gs],
                                        op=ALU.add)
                for k in range(g0, min(g1, KP)):
                    nc.gpsimd.indirect_dma_start(
                        out=hk[k], out_offset=None, in_=table,
                        in_offset=bass.IndirectOffsetOnAxis(
                            ap=gidx[:, :].bitcast(DT.int32)[:, k:k + 1], axis=0))

            nc.vector.max(out=wk[:, 0:8], in_=vals)
            nc.vector.max_index(out=wp[:, 0:8], in_max=wk[:, 0:8], in_values=vals)
            nc.vector.match_replace(out=vals2, in_to_replace=wk[:, 0:8],
                                    in_values=vals, imm_value=0.0)
            decode_and_gather(0, 8)
            nc.vector.max(out=wk[:, 8:16], in_=vals2)
            nc.vector.max_index(out=wp[:, 8:16], in_max=wk[:, 8:16], in_values=vals2)
            nc.vector.match_replace(out=vals3, in_to_replace=wk[:, 8:16],
                                    in_values=vals2, imm_value=0.0)
            decode_and_gather(8, 16)
            nc.vector.max(out=wk[:, 16:24], in_=vals3)
            nc.vector.max_index(out=wp[:, 16:24], in_max=wk[:, 16:24], in_values=vals3)
            decode_and_gather(16, KP)

            d = cpool.tile([P, KPAD], DT.float32, tag="d")
            nc.vector.memset(d[:, KP:], NEG)
            prod = cpool.tile([P, E], DT.float32, tag="prod")
            for k in range(KP):
                nc.vector.scalar_tensor_tensor(
                    prod, hk[k], 1.0, emb_own,
                    op0=ALU.mult, op1=ALU.mult, accum_out=d[:, k:k + 1])

            # 16th largest exact dot -> threshold mask
            t8a = cpool.tile([P, 8], DT.float32, tag="t8a")
            t8b = cpool.tile([P, 8], DT.float32, tag="t8b")
            d2 = cpool.tile([P, KPAD], DT.float32, tag="d2")
            nc.vector.max(out=t8a, in_=d)
            nc.vector.match_replace(out=d2, in_to_replace=t8a, in_values=d,
                                    imm_value=NEG)
            nc.vector.max(out=t8b, in_=d2)
            # maskp = (1[d >= thr16] - 1) * 1e9   (0 for kept, -1e9 for dropped)
            maskp = cpool.tile([P, KP], DT.float32, tag="maskp")
            nc.vector.tensor_scalar(maskp, d[:, :KP], t8b[:, 7:8], None,
                                    op0=ALU.is_ge)
            nc.vector.tensor_scalar(maskp, maskp, -1.0, 1.0e9,
                                    op0=ALU.add, op1=ALU.mult)

            # ---------------- attention over the 20-candidate pool -------
            # hT chunks: n = k*128 + t, grouped 4 k's per 512-wide chunk
            for g in range(NGR):
                hTs = [work.tile([P, 512], DT.float32r, tag=f"hTs{kb}",
                                 name=f"hTs{kb}")
                       for kb in range(2)]
                for kk in range(4):
                    k = g * 4 + kk
                    for kb in range(2):
                        pt = ps_tr.tile([P, P], DT.float32, tag="tr")
                        nc.tensor.transpose(out=pt,
                                            in_=hk[k][:, kb * P:(kb + 1) * P],
                                            identity=ident)
                        nc.vector.tensor_copy(hTs[kb][:, kk * P:(kk + 1) * P], pt)
                tanhTs = [work.tile([P, 512], DT.float32r, tag=f"tanhTs{eb}",
                                    name=f"tanhTs{eb}")
                          for eb in range(2)]
                for eb in range(2):
                    pta = ps_chunk.tile([P, 512], DT.float32, tag="chunk", name="pta")
                    for kb in range(2):
                        nc.tensor.matmul(pta, a_r[kb][:, eb * P:(eb + 1) * P],
                                         hTs[kb], start=(kb == 0), stop=(kb == 1))
                    nc.scalar.activation(tanhTs[eb], pta, ACT.Tanh)
                psc = ps_chunk.tile([1, 512], DT.float32, tag="chunk", name="psc")
                for eb in range(2):
                    nc.tensor.matmul(psc, b_r[eb], tanhTs[eb],
                                     start=(eb == 0), stop=(eb == 1))
                scs = work.tile([1, 512], DT.float32, tag="scs")
                nc.vector.tensor_copy(scs, psc)
                nc.sync.dma_start(out=scd[:, g * 512:(g + 1) * 512], in_=scs)

            # scores [t, k] <- scd[k*128 + t]
            sct = cpool.tile([P, KP], DT.float32, tag="sct")
            for g in range(NGR):
                nc.sync.dma_start(
                    out=sct[:, g * 4:(g + 1) * 4],
                    in_=scd[:, g * 512:(g + 1) * 512].rearrange(
                        "o (k t) -> (o t) k", t=TPC))

            # masked softmax over k
            nc.vector.tensor_tensor(sct, sct, maskp, op=ALU.add)
            mx = cpool.tile([P, 1], DT.float32, tag="mx")
            nc.vector.reduce_max(mx, sct, axis=mybir.AxisListType.X)
            negmx = cpool.tile([P, 1], DT.float32, tag="negmx")
            nc.vector.tensor_scalar(negmx, mx, -1.0, None, op0=ALU.mult)
            ex = cpool.tile([P, KP], DT.float32, tag="ex")
            nc.scalar.activation(ex, sct, ACT.Exp, bias=negmx[:, :], scale=1.0)
            sm = cpool.tile([P, 1], DT.float32, tag="sm")
            nc.vector.reduce_sum(sm, ex, axis=mybir.AxisListType.X)
            rc = cpool.tile([P, 1], DT.float32, tag="rc")
            nc.vector.reciprocal(rc, sm)
            att = cpool.tile([P, KP], DT.float32, tag="att_w")
            nc.vector.tensor_scalar(att, ex, rc[:, :], None, op0=ALU.mult)

            # out[t, e] = sum_k att[t,k] * h[t,k,e]
            acc = cpool.tile([P, E], DT.float32, tag="acc")
            accB = cpool.tile([P, E], DT.float32, tag="accB")
            term = cpool.tile([P, E], DT.float32, tag="term", bufs=3)
            nc.vector.memset(acc, 0.0)
            nc.vector.memset(accB, 0.0)
            for k in range(KP):
                if k % 2 == 0:
                    nc.vector.scalar_tensor_tensor(
                        acc, hk[k], att[:, k:k + 1], acc,
                        op0=ALU.mult, op1=ALU.add)
                else:
                    tk = cpool.tile([P, E], DT.float32, tag="term", bufs=3,
                                    name="tk")
                    nc.scalar.activation(tk, hk[k], ACT.Copy,
                                         scale=att[:, k:k + 1])
                    nc.vector.tensor_tensor(accB, accB, tk, op=ALU.add)
            nc.vector.tensor_tensor(acc, acc, accB, op=ALU.add)
            nc.sync.dma_start(out=out, in_=acc)

    nc.compile()
    return nc


def get_nc():
    if "v3" not in _BUILD_CACHE:
        _BUILD_CACHE["v3"] = _build()
    return _BUILD_CACHE["v3"]


def kernel(conceptnet_text_vec, table, a, b, topk=16, **_ignored):
    global LAST_RESULTS
    assert int(topk) == TOPK
    tok = np.asarray(conceptnet_text_vec).reshape(NTOK, 1).astype(np.int32)
    table = np.ascontiguousarray(np.asarray(table, dtype=np.float32))
    a = np.ascontiguousarray(np.asarray(a, dtype=np.float32))
    b = np.ascontiguousarray(np.asarray(b, dtype=np.float32)).reshape(E, 1)
    tabT_r = _round12(np.ascontiguousarray(table.T))   # [E, V], f32r-rounded

    nc = get_nc()
    in_maps = []
    for c in range(NCORES):
        in_maps.append({
            "tokidx": tok,
            "tok_own": np.ascontiguousarray(tok[c * TPC:(c + 1) * TPC]),
            "table": table,
            "tabTr": np.ascontiguousarray(tabT_r[:, c * VS:(c + 1) * VS]),
            "amat": a,
            "bvec": b,
        })
    trace = bool(int(os.environ.get("CN_TRACE", "0")))
    res = bass_utils.run_bass_kernel_spmd(nc, in_maps, core_ids=list(range(NCORES)),
                                          trace=trace)
    LAST_RESULTS = res
    outp = np.concatenate([res.results[c]["out"] for c in range(NCORES)], axis=0)
    return outp.reshape(B, L, E)
